# revision 20
# baseline (speedup 1.0000x reference)
"""Bass/Tile kernel for nn_Attn_40424232189956 on 8 trn2 NeuronCores.

GQA attention block: q/k/v proj + rmsnorm + rope + causal attention + out proj.
B=2, T=2048, D=2048, NH=16, NKV=4, HD=128.

Sharding: core c -> (batch b = c//4, kv-group kvg = c%4). Each core owns one
batch's tokens, q heads 4*kvg..4*kvg+3 and kv head kvg; it computes a full
[T, D] partial of the output projection and the host sums the 4 partials per
batch. Unlike the head-only sharding this removes all duplicated k/v
projection work and halves both the x load and the output store traffic.

Performance structure (tuned against the TimelineSim cost model):
- Projections and the output projection run as THREE fp8 (e4m3) DoubleRow
  matmul terms: a8@b8 + da8@b8 + a8@db8, where da/db are the fp8 residuals
  of the bf16-class value. DoubleRow processes two 128-contraction tiles per
  instruction at 0.5 cycles/moving-column in the cost model, so the 3-term
  split costs 0.75x of the bf16 pair while keeping ~bf16 accuracy (the
  dropped da@db term and residual quantization are ~0.1% effects).
  Weights ship 32-scaled (fp8 subnormal cutoff), x unscaled; the rmsnorm
  absorbs the 32x PSUM scale for q/k (EPS bias scaled by 1024), the v copy
  divides by 32, and the oproj staging copy divides by 128 (=4*32, the 4
  from folding 0.25 into the softmax-denominator ones matmul).
- Attention (scores, exp, pv) stays bf16: fp8 there fails the 2e-2 gate.
  Far (fully-causal) key-block pairs share one [128,2,512] PSUM stile and a
  single [128,1024] exp; diagonal blocks keep windowed exps + gpsimd
  affine_select masking. exp carries a -2 bias (cancelled by the softmax
  normalization) so pj and the fp8 pads stay inside e4m3 range.
- Softmax denominators: far pad pairs are fp8 and summed with a DoubleRow
  ones matmul (4x cheaper); diagonal pads stay bf16 ones-matmuls so the
  short early rows keep bf16-accurate denominators.
- Normalized attention output is split y8+dy8 (fp8 + fp8 residual) on
  DVE/gpsimd for the 3-term oproj; r64 multiplies by 4/denominator.
- oproj units (16 per group) are interleaved into the NEXT group's first
  attention pass so the PE never waits on the normalize chain; the last
  group alternates two PSUM pools and stores per-128-token-block.
- Engine balance: ACT = qsb/v copies, Square, Sqrt, exp, half the staging
  copies (sqrt and exp eras separated to minimize act-table loads); DVE =
  rope mixes (bf16 2x), qn, reciprocals, pad adds, t-muls, other staging
  copies; gpsimd = one rope mix, affine_select, y8 quantize + dy residual.
"""

import numpy as np

B, T, D = 2, 2048, 2048
NH, NKV = 16, 4
HD = 128
NCORES = 8
HPC = 4               # q heads per core
NKT = D // 128        # 16 contraction tiles for projections
CHUNK = 512
EPS = float(np.finfo(np.float32).eps)
EXP_BIAS = -2.0
WSCALE = 32.0


def _rope_tables():
    # Matches reference.rotary_tables for T=2048 > tsl=1024 (NTK branch).
    hd = np.float32(HD)
    ar = (np.arange(0, HD, 2, dtype=np.float32) / hd).astype(np.float32)
    expo = np.power(np.float32(HD / (HD - 2.0)), ar, dtype=np.float32)
    inv = (np.float32(1.0)
           / (np.float32(10000.0)
              * np.power(np.float32(T / 1024.0), expo, dtype=np.float32)))
    f = np.outer(np.arange(T, dtype=np.float32), inv.astype(np.float32))
    return (np.cos(f).astype(np.float32).T.copy(),
            np.sin(f).astype(np.float32).T.copy())  # [64, T] hd-major


def _build_program():
    import concourse.bass as bass
    import concourse.mybir as mybir
    import concourse.tile as tile
    from concourse import bacc

    f32 = mybir.dt.float32
    f32r = mybir.dt.float32r
    bf16 = mybir.dt.bfloat16
    f8 = mybir.dt.float8e4
    DR = mybir.MatmulPerfMode.DoubleRow
    nc = bacc.Bacc("TRN2", target_bir_lowering=False)

    # x layout [kg, base/resid, ko-within-group, p, t] so (base/resid, ko)
    # merge into one 3D-balanceable access pattern per chunk DMA.
    xcat = nc.dram_tensor("xcat", [4, 2, 4, 128, T], f8,
                          kind="ExternalInput")
    qwcat = nc.dram_tensor("qwcat", [2, D, HPC * HD], f8, kind="ExternalInput")
    kwcat = nc.dram_tensor("kwcat", [2, D, HD], f8, kind="ExternalInput")
    vwcat = nc.dram_tensor("vwcat", [2, D, HD], f8, kind="ExternalInput")
    owcat = nc.dram_tensor("owcat", [2, HPC * HD, D], f8, kind="ExternalInput")
    csd = nc.dram_tensor("csd", [128, T], bf16, kind="ExternalInput")
    csd2 = nc.dram_tensor("csd2", [128, T], bf16, kind="ExternalInput")
    normod = nc.dram_tensor("normod", [128, 5, 128], f32r, kind="ExternalInput")
    normbd = nc.dram_tensor("normbd", [128, 5], f32, kind="ExternalInput")
    # [g, p, tb, d]; host reassembles rows g*512 + tb*128 + p.
    outd = nc.dram_tensor("o", [4, 128, 4, D], bf16, kind="ExternalOutput")

    xr = xcat.rearrange("kg two ko p t -> p kg two ko t")
    qwr = qwcat.rearrange("two (ko p) m -> p two ko m", p=128)
    kwr = kwcat.rearrange("two (ko p) m -> p two ko m", p=128)
    vwr = vwcat.rearrange("two (ko p) m -> p two ko m", p=128)
    owr = owcat.rearrange("two (h p) n -> p two h n", p=128)

    sq_ = mybir.ActivationFunctionType.Square
    ln_ = mybir.ActivationFunctionType.Ln
    exp_ = mybir.ActivationFunctionType.Exp
    copy_ = mybir.ActivationFunctionType.Copy

    with tile.TileContext(nc) as tc:
        with (
            tc.tile_pool(name="wpool", bufs=1) as wpool,
            tc.tile_pool(name="xpool", bufs=2) as xpool,
            tc.tile_pool(name="big", bufs=1) as big,
            tc.tile_pool(name="qsbp", bufs=10) as qsbp,
            tc.tile_pool(name="vtp", bufs=2) as vtp,
            tc.tile_pool(name="ntmp", bufs=2) as ntmp,
            tc.tile_pool(name="ntm2", bufs=1) as ntm2,
            tc.tile_pool(name="pjp", bufs=6) as pjp,
            tc.tile_pool(name="padp", bufs=3) as padp,
            tc.tile_pool(name="padd", bufs=3) as paddp,
            tc.tile_pool(name="ttp", bufs=2) as ttp,
            tc.tile_pool(name="y8p", bufs=2) as y8p,
            tc.tile_pool(name="dy8p", bufs=2) as dy8p,
            tc.tile_pool(name="atmp", bufs=2) as atmp,
            tc.tile_pool(name="obp", bufs=1) as obp,
            tc.tile_pool(name="pst2", bufs=2, space="PSUM") as pst2,
            tc.tile_pool(name="py", bufs=2, space="PSUM") as py,
            tc.tile_pool(name="psm", bufs=1, space="PSUM") as psm,
            tc.tile_pool(name="pox", bufs=1, space="PSUM") as pox,
        ):
            # ---- resident weights / tables (DMAs emitted lazily below) ----
            qw_s = wpool.tile([128, 2, NKT, HPC * HD], f8)
            kw_s = wpool.tile([128, 2, NKT, HD], f8)
            vw_s = wpool.tile([128, 2, NKT, HD], f8)
            ow_s = wpool.tile([128, 2, HPC, D], f8)
            cs_s = wpool.tile([128, T], bf16)   # rows 0:64 cos, 64:128 sin
            cs2_s = wpool.tile([128, T], bf16)  # rows 0:64 sin, 64:128 cos
            normo_s = wpool.tile([128, 5, 128], f32r)
            normb_s = wpool.tile([128, 5], f32)
            ones_bf = wpool.tile([128, 64], bf16)
            nc.vector.memset(ones_bf[:], 0.25)
            # masked DR ones: rhs slot 0 (head 0's pad) -> rows 0:64, slot 1
            # (head 1's pad) -> rows 64:128, one DoubleRow matmul for both.
            ones8 = wpool.tile([128, 2, 128], f8)
            nc.vector.memset(ones8[:], 0.0)
            nc.vector.memset(ones8[:, 0, 0:64], 0.25)
            nc.vector.memset(ones8[:, 1, 64:128], 0.25)
            ebias = wpool.tile([128, 1], f32)
            nc.vector.memset(ebias[:], EXP_BIAS)
            zbias = wpool.tile([128, 1], f32)
            nc.vector.memset(zbias[:], 0.0)

            qT = big.tile([128, HPC, T], bf16)
            kT = big.tile([128, T], bf16)
            vtok = big.tile([128, T], bf16)

            def wdma_qw():
                nc.sync.dma_start(qw_s[:, 0, 0:1], qwr[:, 0, 0:1])
                nc.sync.dma_start(qw_s[:, 0, 1:8], qwr[:, 0, 1:8])
                nc.sync.dma_start(qw_s[:, 0, 8:16], qwr[:, 0, 8:16])
                nc.sync.dma_start(qw_s[:, 1], qwr[:, 1])

            def wdma_kw():
                nc.sync.dma_start(kw_s[:], kwr[:])

            def wdma_vw():
                nc.sync.dma_start(vw_s[:], vwr[:])

            def wdma_tables():
                nc.sync.dma_start(cs_s[:], csd[:])
                nc.sync.dma_start(cs2_s[:], csd2[:])
                nc.sync.dma_start(normo_s[:], normod[:])
                nc.sync.dma_start(normb_s[:], normbd[:])

            def wdma_ow():
                nc.sync.dma_start(ow_s[:, 0], owr[:, 0])
                nc.sync.dma_start(ow_s[:, 1], owr[:, 1])

            def norm_math(qsb, ni, dst, pos0):
                """qsb: sbuf f32 [128 feat, 512 tok] at 32x scale; ni: 0..3 q
                heads, 4 k. dst: [128, 512] bf16 slice of qT/kT. rmsnorm
                (qg & attn scale folded, 32x absorbed) + rope, hd-major."""
                sq = ntmp.tile([128, CHUNK], f32r, tag="sq")
                nc.scalar.activation(out=sq[:], in_=qsb[:], func=sq_)
                nb = pox.tile([128, CHUNK], f32, tag="pox",
                              name=f"nb_{ni}_{pos0}")
                nc.tensor.matmul(nb[:], normo_s[:, ni, :], sq[:],
                                 start=True, stop=True)
                # rfac = rsqrt(nb + eps) = exp(-0.5*ln(.)): keeps every ACT
                # func in the natural_log_exp table set (no table reloads)
                # and needs no DVE reciprocal.
                rs = ntmp.tile([128, CHUNK], f32, tag="rs")
                nc.scalar.activation(out=rs[:], in_=nb[:], func=ln_,
                                     bias=normb_s[:, ni:ni + 1], scale=1.0)
                rfac = ntmp.tile([128, CHUNK], f32, tag="rfac")
                nc.scalar.activation(out=rfac[:], in_=rs[:], func=exp_,
                                     bias=zbias[:], scale=-0.5)
                qn = ntmp.tile([128, CHUNK], bf16, tag="qn")
                nc.vector.tensor_mul(qn[:], qsb[:], rfac[:])
                cs = cs_s[0:64, pos0:pos0 + CHUNK]       # cos @ base 0
                sn = cs_s[64:128, pos0:pos0 + CHUNK]     # sin @ base 64
                sn0 = cs2_s[0:64, pos0:pos0 + CHUNK]     # sin @ base 0
                cs64 = cs2_s[64:128, pos0:pos0 + CHUNK]  # cos @ base 64
                t1 = ntm2.tile([64, CHUNK], bf16, tag="ta")
                t2 = ntm2.tile([64, CHUNK], bf16, tag="tb")
                nc.gpsimd.tensor_mul(t1[:], qn[0:64, :], cs)
                nc.vector.tensor_mul(t2[:], qn[64:128, :], sn)
                nc.vector.tensor_add(dst[0:64, :], t1[:], t2[:])
                t3 = ntm2.tile([64, CHUNK], bf16, tag="tc")
                t4 = ntm2.tile([64, CHUNK], bf16, tag="tb")
                nc.vector.tensor_mul(t3[:], qn[0:64, :], sn0)
                nc.vector.tensor_mul(t4[:], qn[64:128, :], cs64)
                nc.vector.tensor_sub(dst[64:128, :], t4[:], t3[:])

            def emit_xdmas(ci, first=False, extra=None):
                t0 = ci * CHUNK
                xt = xpool.tile([128, 4, 2, 4, CHUNK], f8, tag="xt",
                                name=f"xt_{ci}")
                if first:
                    # split by kg so the first matmuls start early, with the
                    # weight DMAs spliced between
                    for kg in range(4):
                        nc.sync.dma_start(
                            xt[:, kg], xr[:, kg, :, :, t0:t0 + CHUNK])
                        if kg == 0:
                            wdma_qw()
                        elif kg == 1:
                            wdma_kw()
                        elif kg == 2:
                            wdma_vw()
                else:
                    nc.sync.dma_start(xt[:], xr[:, :, :, :, t0:t0 + CHUNK])
                if extra is not None:
                    extra()
                return xt

            def proj_pass(xt, w_s, blks, pos0, tagn):
                """One 2-output-block pass of the 3-term fp8 projection.
                blks: two col0 stationary selectors into w_s's last dim.
                Returns the [128,2,CHUNK] psum tile."""
                pq = pst2.tile([128, 2, CHUNK], f32, tag="pst2",
                               name=f"pq_{tagn}_{pos0}")
                for term in range(3):
                    xt_i = 1 if term == 1 else 0
                    w_i = 1 if term == 2 else 0
                    for pr in range(8):
                        rhs = xt[:, pr // 2, xt_i,
                                 2 * (pr % 2):2 * (pr % 2) + 2, :]
                        st = (term == 0 and pr == 0)
                        sp = (term == 2 and pr == 7)
                        for i, c0 in enumerate(blks):
                            lhsT = w_s[:, w_i, 2 * pr:2 * pr + 2,
                                       c0:c0 + 128]
                            nc.tensor.matmul(pq[:, i, :], lhsT, rhs,
                                             start=st, stop=sp,
                                             perf_mode=DR,
                                             skip_group_check=True)
                return pq

            def proj_chunk(ci, first=False, extra=None):
                pos0 = ci * CHUNK
                xt = emit_xdmas(ci, first=first, extra=extra)
                qsbs = []
                pq = proj_pass(xt, qw_s, (0, 128), pos0, "q01")
                for i in range(2):
                    qsb = qsbp.tile([128, CHUNK], bf16, tag="qsb",
                                    name=f"qsb{i}_{ci}")
                    nc.scalar.copy(out=qsb[:], in_=pq[:, i, :])
                    qsbs.append(qsb)
                pq = proj_pass(xt, qw_s, (256, 384), pos0, "q23")
                for i in range(2):
                    qsb = qsbp.tile([128, CHUNK], bf16, tag="qsb",
                                    name=f"qsb{i+2}_{ci}")
                    nc.scalar.copy(out=qsb[:], in_=pq[:, i, :])
                    qsbs.append(qsb)
                pkv = pst2.tile([128, 2, CHUNK], f32, tag="pst2",
                                name=f"pkv_{ci}")
                for term in range(3):
                    xt_i = 1 if term == 1 else 0
                    w_i = 1 if term == 2 else 0
                    for pr in range(8):
                        rhs = xt[:, pr // 2, xt_i,
                                 2 * (pr % 2):2 * (pr % 2) + 2, :]
                        st = (term == 0 and pr == 0)
                        sp = (term == 2 and pr == 7)
                        nc.tensor.matmul(pkv[:, 0, :],
                                         kw_s[:, w_i, 2 * pr:2 * pr + 2, :],
                                         rhs, start=st, stop=sp,
                                         perf_mode=DR, skip_group_check=True)
                        nc.tensor.matmul(pkv[:, 1, :],
                                         vw_s[:, w_i, 2 * pr:2 * pr + 2, :],
                                         rhs, start=st, stop=sp,
                                         perf_mode=DR, skip_group_check=True)
                qsbk = qsbp.tile([128, CHUNK], bf16, tag="qsb",
                                 name=f"qsbk_{ci}")
                nc.scalar.copy(out=qsbk[:], in_=pkv[:, 0, :])
                vtmp = vtp.tile([128, CHUNK], bf16, tag="vtmp",
                                name=f"vtmp_{ci}")
                nc.scalar.activation(out=vtmp[:], in_=pkv[:, 1, :],
                                     func=copy_, scale=1.0 / WSCALE)

                def finish():
                    for h in range(4):
                        norm_math(qsbs[h], h, qT[:, h, pos0:pos0 + CHUNK],
                                  pos0)
                    norm_math(qsbk, 4, kT[:, pos0:pos0 + CHUNK], pos0)
                    for tb in range(4):
                        dst0 = pos0 + tb * 128
                        nc.sync.dma_start_transpose(
                            vtok[:, dst0:dst0 + 128],
                            vtmp[:, tb * 128:(tb + 1) * 128])
                return finish

            y8s = {}

            def attn_pass(g, hp, fillers=()):
                """Scores/exp/pads/pv/sums for heads (2hp, 2hp+1) of query
                group g; finalize writes y8/dy8. fillers: emission closures
                (previous group's oproj units) spread across the pass."""
                kg = 4 * (g + 1)
                q0 = g * CHUNK
                heads = (2 * hp, 2 * hp + 1)
                if hp == 0:
                    y8s[g] = (y8p.tile([128, HPC, CHUNK], f8, tag="y8",
                                       name=f"y8_{g}"),
                              dy8p.tile([128, HPC, CHUNK], f8, tag="dy8",
                                        name=f"dy8_{g}"))
                y8t, dy8t = y8s[g]
                ys = {}
                for i, h in enumerate(heads):
                    ys[i] = py.tile([128, CHUNK], f32, tag="py",
                                    name=f"y_{g}_{hp}_{i}")
                smt = psm.tile([128, CHUNK], f32, tag="psm",
                               name=f"sm_{g}_{hp}")
                sms = [smt[0:64, :], smt[64:128, :]]

                n_far = 2 * g            # far j-pairs per head
                stepn = [0]
                n_steps = (n_far + 2) * 2 + 2
                fi = [0]

                def fill():
                    stepn[0] += 1
                    want = min(len(fillers),
                               (stepn[0] * len(fillers)) // n_steps + 1)
                    while fi[0] < want:
                        fillers[fi[0]]()
                        fi[0] += 1

                pend_pv = []
                pend_sums = []
                sums_started = [False, False]

                def emit_pv(unit):
                    jj0, pjt, i, c0s = unit
                    for jj in range(2):
                        j = jj0 + jj
                        nc.tensor.matmul(
                            ys[i][:, c0s[jj]:],
                            vtok[:, j * 128:(j + 1) * 128],
                            pjt[:, jj, c0s[jj]:],
                            start=(j == 0), stop=(j == kg - 1),
                            skip_group_check=True)

                def emit_sums(unit):
                    kind, i, op, c0 = unit
                    if kind == "dr":
                        # writes both heads' 64-row ranges at once
                        st = not (sums_started[0] or sums_started[1])
                        sums_started[0] = sums_started[1] = True
                        nc.tensor.matmul(smt[:, c0:], ones8[:],
                                         op[:, :, c0:], start=st, stop=False,
                                         perf_mode=DR, skip_group_check=True)
                    else:
                        st = not sums_started[i]
                        sums_started[i] = True
                        nc.tensor.matmul(sms[i][:, c0:], ones_bf[:],
                                         op[:, c0:], start=st,
                                         stop=(kind == "last"),
                                         skip_group_check=True)

                def drain(pv_keep=1, sums_keep=1):
                    while len(pend_pv) > pv_keep:
                        emit_pv(pend_pv.pop(0))
                    while len(pend_sums) > sums_keep:
                        emit_sums(pend_sums.pop(0))

                padq = {}
                # far pairs: j = 2p, 2p+1; fully causal, full query width
                for p in range(n_far):
                    for i, h in enumerate(heads):
                        st2 = pst2.tile([128, 2, CHUNK], f32, tag="pst2",
                                        name=f"st_{g}_{hp}_{p}_{i}")
                        for jj in range(2):
                            j = 2 * p + jj
                            nc.tensor.matmul(
                                st2[:, jj, :],
                                kT[:, j * 128:(j + 1) * 128],
                                qT[:, h, q0:q0 + CHUNK],
                                start=True, stop=True,
                                skip_group_check=True)
                        drain()
                        pjt = pjp.tile([128, 2, CHUNK], bf16, tag="pj",
                                       name=f"pj_{g}_{hp}_{p}_{i}")
                        nc.scalar.activation(out=pjt[:], in_=st2[:],
                                             func=exp_, bias=ebias[:],
                                             scale=1.0)
                        if i == 0:
                            padq[p] = padp.tile([128, 2, CHUNK], f8,
                                                tag="padq",
                                                name=f"pq_{g}_{hp}_{p}")
                        nc.vector.tensor_add(padq[p][:, i, :],
                                             pjt[:, 0, :], pjt[:, 1, :])
                        if i == 1:
                            pend_sums.append(("dr", 0, padq[p], 0))
                        pend_pv.append((2 * p, pjt, i, (0, 0)))
                        fill()
                # diagonal pairs: j = 4g+2dp, 4g+2dp+1
                for dp in range(2):
                    for i, h in enumerate(heads):
                        j0 = 4 * g + 2 * dp
                        c00, c01 = 256 * dp, 256 * dp + 128
                        st2 = pst2.tile([128, 2, CHUNK], f32, tag="pst2",
                                        name=f"std_{g}_{hp}_{dp}_{i}")
                        nc.tensor.matmul(st2[:, 0, c00:],
                                         kT[:, j0 * 128:(j0 + 1) * 128],
                                         qT[:, h, q0 + c00:q0 + CHUNK],
                                         start=True, stop=True,
                                         skip_group_check=True)
                        nc.tensor.matmul(st2[:, 1, c01:],
                                         kT[:, (j0 + 1) * 128:(j0 + 2) * 128],
                                         qT[:, h, q0 + c01:q0 + CHUNK],
                                         start=True, stop=True,
                                         skip_group_check=True)
                        drain()
                        pjt = pjp.tile([128, 2, CHUNK], bf16, tag="pj",
                                       name=f"pjd_{g}_{hp}_{dp}_{i}")
                        nc.scalar.activation(out=pjt[:, 0, c00:],
                                             in_=st2[:, 0, c00:],
                                             func=exp_, bias=ebias[:],
                                             scale=1.0)
                        nc.scalar.activation(out=pjt[:, 1, c01:],
                                             in_=st2[:, 1, c01:],
                                             func=exp_, bias=ebias[:],
                                             scale=1.0)
                        nc.vector.memset(pjt[:, 1, c00:c01], 0.0)
                        for jj, cc in ((0, c00), (1, c01)):
                            nc.gpsimd.affine_select(
                                out=pjt[:, jj, cc:cc + 128],
                                in_=pjt[:, jj, cc:cc + 128],
                                pattern=[[1, 128]],
                                compare_op=mybir.AluOpType.is_ge,
                                fill=0.0,
                                base=0,
                                channel_multiplier=-1)
                        pdd = paddp.tile([128, CHUNK], bf16, tag="padd",
                                         name=f"pd_{g}_{hp}_{dp}_{i}")
                        nc.vector.tensor_add(pdd[:, c00:], pjt[:, 0, c00:],
                                             pjt[:, 1, c00:])
                        pend_sums.append(("last" if dp == 1 else "bf",
                                          i, pdd, c00))
                        pend_pv.append((j0, pjt, i, (c00, c01)))
                        fill()
                drain(0, 0)
                # finalize: r64 = 4/denominator; y8/dy8 split for oproj
                for i, h in enumerate(heads):
                    r64 = atmp.tile([64, CHUNK], f32, tag="r64",
                                    name=f"r_{g}_{hp}_{i}")
                    nc.vector.reciprocal(r64[:], sms[i])
                    t = ttp.tile([128, CHUNK], f32, tag="t",
                                 name=f"t_{g}_{hp}_{i}")
                    for half in range(2):
                        p0 = 64 * half
                        tsl = t[p0:p0 + 64, :]
                        nc.vector.tensor_mul(tsl, ys[i][p0:p0 + 64, :],
                                             r64[:])
                        y8sl = y8t[p0:p0 + 64, h, :]
                        if g == 3:
                            # tail: DVE is idle and faster than the Q7 chain
                            nc.vector.tensor_copy(out=y8sl, in_=tsl)
                            nc.vector.tensor_sub(dy8t[p0:p0 + 64, h, :],
                                                 tsl, y8sl)
                        else:
                            nc.gpsimd.tensor_copy(out=y8sl, in_=tsl)
                            nc.gpsimd.tensor_sub(dy8t[p0:p0 + 64, h, :],
                                                 tsl, y8sl)
                    fill()
                while fi[0] < len(fillers):
                    fillers[fi[0]]()
                    fi[0] += 1

            def oproj_units(g, alt=False, split_store=False):
                """16 oproj unit closures + final store; 3-term fp8
                DoubleRow over head pairs. alt: alternate pox/py pools
                (tail group, attention pools free)."""
                obuf = obp.tile([128, 4, D], bf16, tag="obuf",
                                name=f"ob_{g}")
                y8t, dy8t = y8s[g]
                units = []

                def unit(tb, oc, n):
                    def go():
                        pool = py if (alt and n % 2 == 1) else pox
                        po = pool.tile([128, CHUNK], f32,
                                       tag="py" if (alt and n % 2 == 1)
                                       else "pox",
                                       name=f"op_{g}_{tb}_{oc}")
                        first, last = True, None
                        mms = []
                        for hp in range(2):
                            h0 = 2 * hp
                            for term in range(3):
                                ysl = dy8t if term == 1 else y8t
                                w_i = 1 if term == 2 else 0
                                mms.append((
                                    ysl[:, h0:h0 + 2,
                                        tb * 128:(tb + 1) * 128],
                                    ow_s[:, w_i, h0:h0 + 2,
                                         oc * CHUNK:(oc + 1) * CHUNK]))
                        for mi, (lhsT, rhs) in enumerate(mms):
                            nc.tensor.matmul(po[:], lhsT, rhs,
                                             start=(mi == 0),
                                             stop=(mi == len(mms) - 1),
                                             perf_mode=DR,
                                             skip_group_check=True)
                        dst = obuf[:, tb, oc * CHUNK:(oc + 1) * CHUNK]
                        if n % 2 == 0:
                            nc.scalar.activation(out=dst, in_=po[:],
                                                 func=copy_,
                                                 scale=1.0 / 128.0)
                        else:
                            nc.vector.tensor_scalar_mul(dst, po[:],
                                                        1.0 / 128.0)
                        if split_store and oc == 3:
                            nc.sync.dma_start(outd[g, :, tb, :],
                                              obuf[:, tb, :])
                    return go

                n = 0
                for tb in range(4):
                    for oc in range(4):
                        units.append(unit(tb, oc, n))
                        n += 1
                if not split_store:
                    def store():
                        nc.sync.dma_start(outd[g], obuf[:])
                    units.append(store)
                return units

            # ---- schedule ----
            f0 = proj_chunk(0, first=True)
            f1 = proj_chunk(1, extra=wdma_tables)
            f0()
            f2 = proj_chunk(2, extra=wdma_ow)
            f1()
            attn_pass(0, 0)
            f3 = proj_chunk(3)
            attn_pass(0, 1)
            attn_pass(1, 0, fillers=oproj_units(0))
            f2()
            attn_pass(1, 1)
            f3()
            attn_pass(2, 0, fillers=oproj_units(1))
            attn_pass(2, 1)
            attn_pass(3, 0, fillers=oproj_units(2))
            attn_pass(3, 1)
            for u in oproj_units(3, alt=True, split_store=True):
                u()

    nc.compile()
    return nc


_CACHED = {}
LAST_EXEC_NS = None


def _run(nc, in_maps, **kwargs):
    from concourse.bass_utils import run_bass_kernel_spmd
    return run_bass_kernel_spmd(nc, in_maps, core_ids=list(range(NCORES)),
                                **kwargs)


def _make_in_maps(x, qw, kw, vw, ow, qg):
    import ml_dtypes
    bf = ml_dtypes.bfloat16
    f8 = ml_dtypes.float8_e4m3

    def split8(a, scale):
        a32 = np.asarray(a, np.float32) * np.float32(scale)
        a8 = a32.astype(f8)
        da = (a32 - a8.astype(np.float32)).astype(f8)
        return np.stack([a8, da])  # [2, ...]

    cosT, sinT = _rope_tables()
    cossin = np.concatenate([cosT, sinT], axis=0).astype(bf)   # [128, T]
    sincos = np.concatenate([sinT, cosT], axis=0).astype(bf)   # [128, T]

    def xlayout(xb):
        s = split8(xb.reshape(T, D).T, 1.0)        # [2, D, T]
        return np.ascontiguousarray(
            s.reshape(2, 4, 4, 128, T).transpose(1, 0, 2, 3, 4))

    xcats = [xlayout(x[b]) for b in range(B)]

    in_maps = []
    for c in range(NCORES):
        b, kvg = c // 4, c % 4
        h0 = HPC * kvg
        qwcat = split8(qw[h0 * HD:(h0 + HPC) * HD, :].T, WSCALE)
        kwcat = split8(kw[kvg * HD:(kvg + 1) * HD, :].T, WSCALE)
        vwcat = split8(vw[kvg * HD:(kvg + 1) * HD, :].T, WSCALE)
        owcat = split8(ow[:, h0 * HD:(h0 + HPC) * HD].T, WSCALE)
        # s_i folds qg gain and 1/sqrt(HD); 32x psum scale cancels in the
        # ratio, only the EPS bias needs the 32^2 factor.
        s = np.array([qg[h0] / np.sqrt(HD), qg[h0 + 1] / np.sqrt(HD),
                      qg[h0 + 2] / np.sqrt(HD), qg[h0 + 3] / np.sqrt(HD),
                      1.0], np.float32)
        normo = np.broadcast_to(
            (1.0 / (HD * s * s))[None, :, None], (128, 5, 128)
        ).astype(np.float32).copy()
        normb = np.broadcast_to(
            (EPS * WSCALE * WSCALE / (s * s))[None, :],
            (128, 5)).astype(np.float32).copy()
        in_maps.append({
            "xcat": xcats[b],
            "qwcat": np.ascontiguousarray(qwcat),
            "kwcat": np.ascontiguousarray(kwcat),
            "vwcat": np.ascontiguousarray(vwcat),
            "owcat": np.ascontiguousarray(owcat),
            "csd": cossin, "csd2": sincos,
            "normod": normo, "normbd": normb,
        })
    return in_maps


def kernel(x, qw, kw, vw, ow, qg):
    global LAST_EXEC_NS
    x = np.ascontiguousarray(x, dtype=np.float32)
    qw = np.asarray(qw, dtype=np.float32)
    kw = np.asarray(kw, dtype=np.float32)
    vw = np.asarray(vw, dtype=np.float32)
    ow = np.asarray(ow, dtype=np.float32)
    qg = np.asarray(qg, dtype=np.float32)

    if "nc" not in _CACHED:
        _CACHED["nc"] = _build_program()
    nc = _CACHED["nc"]

    in_maps = _make_in_maps(x, qw, kw, vw, ow, qg)
    res = _run(nc, in_maps)
    LAST_EXEC_NS = res.exec_time_ns
    out = np.empty((B, T, D), np.float32)
    for b in range(B):
        acc = np.zeros((4, 128, 4, D), np.float32)
        for kvg in range(4):
            acc += res.results[4 * b + kvg]["o"].astype(np.float32)
        # [g, p, tb, d] -> rows g*512 + tb*128 + p
        out[b] = acc.transpose(0, 2, 1, 3).reshape(T, D)
    return np.ascontiguousarray(out)


# revision 22
# speedup vs baseline: 1.2227x; 1.2227x over previous
"""Bass/Tile kernel for nn_Attn_40424232189956 on 8 trn2 NeuronCores.

GQA attention block: q/k/v proj + rmsnorm + rope + causal attention + out proj.
B=2, T=2048, D=2048, NH=16, NKV=4, HD=128.

Sharding: core c -> (batch b = c//4, kv-group kvg = c%4). Each core owns one
batch's tokens, q heads 4*kvg..4*kvg+3 and kv head kvg; it computes a full
[T, D] partial of the output projection and the host sums the 4 partials per
batch. Unlike the head-only sharding this removes all duplicated k/v
projection work and halves both the x load and the output store traffic.

Performance structure (tuned against the TimelineSim cost model):
- Projections and the output projection run as THREE fp8 (e4m3) DoubleRow
  matmul terms: a8@b8 + da8@b8 + a8@db8, where da/db are the fp8 residuals
  of the bf16-class value. DoubleRow processes two 128-contraction tiles per
  instruction at 0.5 cycles/moving-column in the cost model, so the 3-term
  split costs 0.75x of the bf16 pair while keeping ~bf16 accuracy (the
  dropped da@db term and residual quantization are ~0.1% effects).
  Weights ship 32-scaled (fp8 subnormal cutoff), x unscaled; the rmsnorm
  absorbs the 32x PSUM scale for q/k (EPS bias scaled by 1024), the v copy
  divides by 32, and the oproj staging copy divides by 128 (=4*32, the 4
  from folding 0.25 into the softmax-denominator ones matmul).
- Attention (scores, exp, pv) stays bf16: fp8 there fails the 2e-2 gate.
  Far (fully-causal) key-block pairs share one [128,2,512] PSUM stile and a
  single [128,1024] exp; diagonal blocks keep windowed exps + gpsimd
  affine_select masking. exp carries a -2 bias (cancelled by the softmax
  normalization) so pj and the fp8 pads stay inside e4m3 range.
- Softmax denominators: far pad pairs are fp8 and summed with a DoubleRow
  ones matmul (4x cheaper); diagonal pads stay bf16 ones-matmuls so the
  short early rows keep bf16-accurate denominators.
- Normalized attention output is split y8+dy8 (fp8 + fp8 residual) on
  DVE/gpsimd for the 3-term oproj; r64 multiplies by 4/denominator.
- oproj units (16 per group) are interleaved into the NEXT group's first
  attention pass so the PE never waits on the normalize chain; the last
  group alternates two PSUM pools and stores per-128-token-block.
- Engine balance: ACT = qsb/v copies, Square, Sqrt, exp, half the staging
  copies (sqrt and exp eras separated to minimize act-table loads); DVE =
  rope mixes (bf16 2x), qn, reciprocals, pad adds, t-muls, other staging
  copies; gpsimd = one rope mix, affine_select, y8 quantize + dy residual.
"""

import numpy as np

B, T, D = 2, 2048, 2048
NH, NKV = 16, 4
HD = 128
NCORES = 8
HPC = 4               # q heads per core
NKT = D // 128        # 16 contraction tiles for projections
CHUNK = 512
EPS = float(np.finfo(np.float32).eps)
EXP_BIAS = -2.0
WSCALE = 32.0


def _rope_tables():
    # Matches reference.rotary_tables for T=2048 > tsl=1024 (NTK branch).
    hd = np.float32(HD)
    ar = (np.arange(0, HD, 2, dtype=np.float32) / hd).astype(np.float32)
    expo = np.power(np.float32(HD / (HD - 2.0)), ar, dtype=np.float32)
    inv = (np.float32(1.0)
           / (np.float32(10000.0)
              * np.power(np.float32(T / 1024.0), expo, dtype=np.float32)))
    f = np.outer(np.arange(T, dtype=np.float32), inv.astype(np.float32))
    return (np.cos(f).astype(np.float32).T.copy(),
            np.sin(f).astype(np.float32).T.copy())  # [64, T] hd-major


def _build_program():
    import concourse.bass as bass
    import concourse.mybir as mybir
    import concourse.tile as tile
    from concourse import bacc
    import concourse.hw_specs as hw_specs

    # The act-table-load inserter picks the FIRST table set containing each
    # activation func, so a kernel mixing Ln (rmsnorm rsqrt) and Exp
    # (softmax) flip-flops between the 'natural_log' and 'exp_and_others'
    # sets, paying ~1.28us per switch. Every func we use (Ln, Exp, Square,
    # Copy) lives together in 'natural_log_exp_and_others'; empty the other
    # sets (keeping dict order, so canonical act_func_set_ids are unchanged
    # for walrus) to steer the chooser there once.
    if not getattr(bacc.get_activation_tables, "_attn_patched", False):
        _orig_gat = bacc.get_activation_tables

        def _gat(arch):
            t = _orig_gat(arch)
            keep = "natural_log_exp_and_others"
            if keep not in t:
                return t
            return {k: (v if k == keep else frozenset())
                    for k, v in t.items()}

        _gat._attn_patched = True
        bacc.get_activation_tables = _gat
        hw_specs_gat = getattr(hw_specs, "get_activation_tables", None)
        if hw_specs_gat is not None and not getattr(
                hw_specs_gat, "_attn_patched", False):
            hw_specs.get_activation_tables = _gat

    f32 = mybir.dt.float32
    f32r = mybir.dt.float32r
    bf16 = mybir.dt.bfloat16
    f8 = mybir.dt.float8e4
    DR = mybir.MatmulPerfMode.DoubleRow
    nc = bacc.Bacc("TRN2", target_bir_lowering=False)

    # x layout [kg, base/resid, ko-within-group, p, t] so (base/resid, ko)
    # merge into one 3D-balanceable access pattern per chunk DMA.
    xcat = nc.dram_tensor("xcat", [4, 2, 4, 128, T], f8,
                          kind="ExternalInput")
    # weights ship partition-major so each DMA is one contiguous run per
    # partition (512B+ descriptors: full DMA bus bandwidth in the model)
    qwcat = nc.dram_tensor("qwcat", [128, 2, NKT, HPC * HD], f8,
                           kind="ExternalInput")
    kwcat = nc.dram_tensor("kwcat", [128, 2, NKT, HD], f8,
                           kind="ExternalInput")
    vwcat = nc.dram_tensor("vwcat", [128, 2, NKT, HD], f8,
                           kind="ExternalInput")
    owcat = nc.dram_tensor("owcat", [128, 2, HPC, D], f8,
                           kind="ExternalInput")
    csd = nc.dram_tensor("csd", [128, T], bf16, kind="ExternalInput")
    csd2 = nc.dram_tensor("csd2", [128, T], bf16, kind="ExternalInput")
    normod = nc.dram_tensor("normod", [128, 5, 128], f32r, kind="ExternalInput")
    normbd = nc.dram_tensor("normbd", [128, 5], f32, kind="ExternalInput")
    # [g, p, tb, d]; host reassembles rows g*512 + tb*128 + p.
    outd = nc.dram_tensor("o", [4, 128, 4, D], bf16, kind="ExternalOutput")

    xr = xcat.rearrange("kg two ko p t -> p kg two ko t")

    sq_ = mybir.ActivationFunctionType.Square
    ln_ = mybir.ActivationFunctionType.Ln
    exp_ = mybir.ActivationFunctionType.Exp
    copy_ = mybir.ActivationFunctionType.Copy

    with tile.TileContext(nc) as tc:
        with (
            tc.tile_pool(name="wpool", bufs=1) as wpool,
            tc.tile_pool(name="xpool", bufs=2) as xpool,
            tc.tile_pool(name="big", bufs=1) as big,
            tc.tile_pool(name="qsbp", bufs=10) as qsbp,
            tc.tile_pool(name="vtp", bufs=2) as vtp,
            tc.tile_pool(name="ntmp", bufs=2) as ntmp,
            tc.tile_pool(name="ntm2", bufs=1) as ntm2,
            tc.tile_pool(name="pjp", bufs=6) as pjp,
            tc.tile_pool(name="padp", bufs=3) as padp,
            tc.tile_pool(name="padd", bufs=3) as paddp,
            tc.tile_pool(name="ttp", bufs=2) as ttp,
            tc.tile_pool(name="y8p", bufs=2) as y8p,
            tc.tile_pool(name="dy8p", bufs=2) as dy8p,
            tc.tile_pool(name="atmp", bufs=2) as atmp,
            tc.tile_pool(name="obp", bufs=1) as obp,
            tc.tile_pool(name="pst2", bufs=2, space="PSUM") as pst2,
            tc.tile_pool(name="py", bufs=2, space="PSUM") as py,
            tc.tile_pool(name="psm", bufs=1, space="PSUM") as psm,
            tc.tile_pool(name="pox", bufs=1, space="PSUM") as pox,
        ):
            # ---- resident weights / tables (DMAs emitted lazily below) ----
            qw_s = wpool.tile([128, 2, NKT, HPC * HD], f8)
            kw_s = wpool.tile([128, 2, NKT, HD], f8)
            vw_s = wpool.tile([128, 2, NKT, HD], f8)
            ow_s = wpool.tile([128, 2, HPC, D], f8)
            cs_s = wpool.tile([128, T], bf16)   # rows 0:64 cos, 64:128 sin
            cs2_s = wpool.tile([128, T], bf16)  # rows 0:64 sin, 64:128 cos
            normo_s = wpool.tile([128, 5, 128], f32r)
            normb_s = wpool.tile([128, 5], f32)
            ones_bf = wpool.tile([128, 64], bf16)
            nc.vector.memset(ones_bf[:], 0.25)
            # masked DR ones: rhs slot 0 (head 0's pad) -> rows 0:64, slot 1
            # (head 1's pad) -> rows 64:128, one DoubleRow matmul for both.
            ones8 = wpool.tile([128, 2, 128], f8)
            nc.vector.memset(ones8[:], 0.0)
            nc.vector.memset(ones8[:, 0, 0:64], 0.25)
            nc.vector.memset(ones8[:, 1, 64:128], 0.25)
            ebias = wpool.tile([128, 1], f32)
            nc.vector.memset(ebias[:], EXP_BIAS)
            zbias = wpool.tile([128, 1], f32)
            nc.vector.memset(zbias[:], 0.0)

            qT = big.tile([128, HPC, T], bf16)
            kT = big.tile([128, T], bf16)
            vtok = big.tile([128, T], bf16)

            def wdma_qw():
                nc.sync.dma_start(qw_s[:, 0, 0:1], qwcat[:, 0, 0:1])
                nc.sync.dma_start(qw_s[:, 0, 1:8], qwcat[:, 0, 1:8])
                nc.sync.dma_start(qw_s[:, 0, 8:16], qwcat[:, 0, 8:16])

            def wdma_qwr():
                nc.sync.dma_start(qw_s[:, 1, 0:8], qwcat[:, 1, 0:8])
                nc.sync.dma_start(qw_s[:, 1, 8:16], qwcat[:, 1, 8:16])

            def wdma_kw():
                nc.sync.dma_start(kw_s[:, 0], kwcat[:, 0])
                nc.sync.dma_start(vw_s[:, 0], vwcat[:, 0])

            def wdma_vw():
                nc.sync.dma_start(kw_s[:, 1], kwcat[:, 1])
                nc.sync.dma_start(vw_s[:, 1], vwcat[:, 1])

            def wdma_tables():
                nc.sync.dma_start(cs_s[:], csd[:])
                nc.sync.dma_start(cs2_s[:], csd2[:])
                nc.sync.dma_start(normo_s[:], normod[:])
                nc.sync.dma_start(normb_s[:], normbd[:])

            def wdma_ow():
                nc.sync.dma_start(ow_s[:, 0], owcat[:, 0])
                nc.sync.dma_start(ow_s[:, 1], owcat[:, 1])

            def norm_math(qsb, ni, dst, pos0):
                """qsb: sbuf f32 [128 feat, 512 tok] at 32x scale; ni: 0..3 q
                heads, 4 k. dst: [128, 512] bf16 slice of qT/kT. rmsnorm
                (qg & attn scale folded, 32x absorbed) + rope, hd-major."""
                sq = ntmp.tile([128, CHUNK], f32r, tag="sq")
                nc.scalar.activation(out=sq[:], in_=qsb[:], func=sq_)
                nb = pox.tile([128, CHUNK], f32, tag="pox",
                              name=f"nb_{ni}_{pos0}")
                nc.tensor.matmul(nb[:], normo_s[:, ni, :], sq[:],
                                 start=True, stop=True)
                # rfac = rsqrt(nb + eps) = exp(-0.5*ln(.)): keeps every ACT
                # func in the natural_log_exp table set (no table reloads)
                # and needs no DVE reciprocal.
                rs = ntmp.tile([128, CHUNK], f32, tag="rs")
                nc.scalar.activation(out=rs[:], in_=nb[:], func=ln_,
                                     bias=normb_s[:, ni:ni + 1], scale=1.0)
                rfac = ntmp.tile([128, CHUNK], f32, tag="rfac")
                nc.scalar.activation(out=rfac[:], in_=rs[:], func=exp_,
                                     bias=zbias[:], scale=-0.5)
                qn = ntmp.tile([128, CHUNK], bf16, tag="qn")
                nc.vector.tensor_mul(qn[:], qsb[:], rfac[:])
                cs = cs_s[0:64, pos0:pos0 + CHUNK]       # cos @ base 0
                sn = cs_s[64:128, pos0:pos0 + CHUNK]     # sin @ base 64
                sn0 = cs2_s[0:64, pos0:pos0 + CHUNK]     # sin @ base 0
                cs64 = cs2_s[64:128, pos0:pos0 + CHUNK]  # cos @ base 64
                t1 = ntm2.tile([64, CHUNK], bf16, tag="ta")
                t2 = ntm2.tile([64, CHUNK], bf16, tag="tb")
                nc.gpsimd.tensor_mul(t1[:], qn[0:64, :], cs)
                nc.vector.tensor_mul(t2[:], qn[64:128, :], sn)
                nc.vector.tensor_add(dst[0:64, :], t1[:], t2[:])
                t3 = ntm2.tile([64, CHUNK], bf16, tag="tc")
                t4 = ntm2.tile([64, CHUNK], bf16, tag="tb")
                nc.vector.tensor_mul(t3[:], qn[0:64, :], sn0)
                nc.vector.tensor_mul(t4[:], qn[64:128, :], cs64)
                nc.vector.tensor_sub(dst[64:128, :], t4[:], t3[:])

            def emit_xdmas(ci, first=False, extra=None):
                t0 = ci * CHUNK
                xt = xpool.tile([128, 4, 2, 4, CHUNK], f8, tag="xt",
                                name=f"xt_{ci}")
                if first:
                    # split by kg so the first matmuls start early, with the
                    # weight DMAs spliced between
                    for kg in range(4):
                        nc.sync.dma_start(
                            xt[:, kg], xr[:, kg, :, :, t0:t0 + CHUNK])
                        if kg == 0:
                            wdma_qw()
                        elif kg == 1:
                            wdma_qwr()
                        elif kg == 2:
                            wdma_kw()
                        else:
                            wdma_vw()
                else:
                    nc.sync.dma_start(xt[:], xr[:, :, :, :, t0:t0 + CHUNK])
                if extra is not None:
                    extra()
                return xt

            def proj_pass(xt, w_s, blks, pos0, tagn):
                """One 2-output-block pass of the 3-term fp8 projection.
                blks: two col0 stationary selectors into w_s's last dim.
                Returns the [128,2,CHUNK] psum tile."""
                pq = pst2.tile([128, 2, CHUNK], f32, tag="pst2",
                               name=f"pq_{tagn}_{pos0}")
                for term in range(3):
                    xt_i = 1 if term == 1 else 0
                    w_i = 1 if term == 2 else 0
                    for pr in range(8):
                        rhs = xt[:, pr // 2, xt_i,
                                 2 * (pr % 2):2 * (pr % 2) + 2, :]
                        st = (term == 0 and pr == 0)
                        sp = (term == 2 and pr == 7)
                        for i, c0 in enumerate(blks):
                            lhsT = w_s[:, w_i, 2 * pr:2 * pr + 2,
                                       c0:c0 + 128]
                            nc.tensor.matmul(pq[:, i, :], lhsT, rhs,
                                             start=st, stop=sp,
                                             perf_mode=DR,
                                             skip_group_check=True)
                return pq

            def proj_chunk(ci, first=False, extra=None):
                pos0 = ci * CHUNK
                xt = emit_xdmas(ci, first=first, extra=extra)
                qsbs = []
                pq = proj_pass(xt, qw_s, (0, 128), pos0, "q01")
                for i in range(2):
                    qsb = qsbp.tile([128, CHUNK], bf16, tag="qsb",
                                    name=f"qsb{i}_{ci}")
                    nc.scalar.copy(out=qsb[:], in_=pq[:, i, :])
                    qsbs.append(qsb)
                pq = proj_pass(xt, qw_s, (256, 384), pos0, "q23")
                for i in range(2):
                    qsb = qsbp.tile([128, CHUNK], bf16, tag="qsb",
                                    name=f"qsb{i+2}_{ci}")
                    nc.scalar.copy(out=qsb[:], in_=pq[:, i, :])
                    qsbs.append(qsb)
                pkv = pst2.tile([128, 2, CHUNK], f32, tag="pst2",
                                name=f"pkv_{ci}")
                for term in range(3):
                    xt_i = 1 if term == 1 else 0
                    w_i = 1 if term == 2 else 0
                    for pr in range(8):
                        rhs = xt[:, pr // 2, xt_i,
                                 2 * (pr % 2):2 * (pr % 2) + 2, :]
                        st = (term == 0 and pr == 0)
                        sp = (term == 2 and pr == 7)
                        nc.tensor.matmul(pkv[:, 0, :],
                                         kw_s[:, w_i, 2 * pr:2 * pr + 2, :],
                                         rhs, start=st, stop=sp,
                                         perf_mode=DR, skip_group_check=True)
                        nc.tensor.matmul(pkv[:, 1, :],
                                         vw_s[:, w_i, 2 * pr:2 * pr + 2, :],
                                         rhs, start=st, stop=sp,
                                         perf_mode=DR, skip_group_check=True)
                qsbk = qsbp.tile([128, CHUNK], bf16, tag="qsb",
                                 name=f"qsbk_{ci}")
                nc.scalar.copy(out=qsbk[:], in_=pkv[:, 0, :])
                vtmp = vtp.tile([128, CHUNK], bf16, tag="vtmp",
                                name=f"vtmp_{ci}")
                nc.scalar.activation(out=vtmp[:], in_=pkv[:, 1, :],
                                     func=copy_, scale=1.0 / WSCALE)

                def finish():
                    for h in range(4):
                        norm_math(qsbs[h], h, qT[:, h, pos0:pos0 + CHUNK],
                                  pos0)
                    norm_math(qsbk, 4, kT[:, pos0:pos0 + CHUNK], pos0)
                    for tb in range(4):
                        dst0 = pos0 + tb * 128
                        nc.sync.dma_start_transpose(
                            vtok[:, dst0:dst0 + 128],
                            vtmp[:, tb * 128:(tb + 1) * 128])
                return finish

            y8s = {}

            def attn_pass(g, hp, fillers=()):
                """Scores/exp/pads/pv/sums for heads (2hp, 2hp+1) of query
                group g; finalize writes y8/dy8. fillers: emission closures
                (previous group's oproj units) spread across the pass."""
                kg = 4 * (g + 1)
                q0 = g * CHUNK
                heads = (2 * hp, 2 * hp + 1)
                if hp == 0:
                    y8s[g] = (y8p.tile([128, HPC, CHUNK], f8, tag="y8",
                                       name=f"y8_{g}"),
                              dy8p.tile([128, HPC, CHUNK], f8, tag="dy8",
                                        name=f"dy8_{g}"))
                y8t, dy8t = y8s[g]
                ys = {}
                for i, h in enumerate(heads):
                    ys[i] = py.tile([128, CHUNK], f32, tag="py",
                                    name=f"y_{g}_{hp}_{i}")
                smt = psm.tile([128, CHUNK], f32, tag="psm",
                               name=f"sm_{g}_{hp}")
                sms = [smt[0:64, :], smt[64:128, :]]

                n_far = 2 * g            # far j-pairs per head
                stepn = [0]
                n_steps = (n_far + 2) * 2 + 2
                fi = [0]

                def fill():
                    stepn[0] += 1
                    want = min(len(fillers),
                               (stepn[0] * len(fillers)) // n_steps + 1)
                    while fi[0] < want:
                        fillers[fi[0]]()
                        fi[0] += 1

                pend_pv = []
                pend_sums = []
                sums_started = [False, False]

                def emit_pv(unit):
                    jj0, pjt, i, c0s = unit
                    for jj in range(2):
                        j = jj0 + jj
                        nc.tensor.matmul(
                            ys[i][:, c0s[jj]:],
                            vtok[:, j * 128:(j + 1) * 128],
                            pjt[:, jj, c0s[jj]:],
                            start=(j == 0), stop=(j == kg - 1),
                            skip_group_check=True)

                def emit_sums(unit):
                    kind, i, op, c0 = unit
                    if kind == "dr":
                        # writes both heads' 64-row ranges at once
                        st = not (sums_started[0] or sums_started[1])
                        sums_started[0] = sums_started[1] = True
                        nc.tensor.matmul(smt[:, c0:], ones8[:],
                                         op[:, :, c0:], start=st, stop=False,
                                         perf_mode=DR, skip_group_check=True)
                    else:
                        st = not sums_started[i]
                        sums_started[i] = True
                        nc.tensor.matmul(sms[i][:, c0:], ones_bf[:],
                                         op[:, c0:], start=st,
                                         stop=(kind == "last"),
                                         skip_group_check=True)

                def drain(pv_keep=1, sums_keep=1):
                    while len(pend_pv) > pv_keep:
                        emit_pv(pend_pv.pop(0))
                    while len(pend_sums) > sums_keep:
                        emit_sums(pend_sums.pop(0))

                padq = {}
                # far pairs: j = 2p, 2p+1; fully causal, full query width
                for p in range(n_far):
                    for i, h in enumerate(heads):
                        st2 = pst2.tile([128, 2, CHUNK], f32, tag="pst2",
                                        name=f"st_{g}_{hp}_{p}_{i}")
                        for jj in range(2):
                            j = 2 * p + jj
                            nc.tensor.matmul(
                                st2[:, jj, :],
                                kT[:, j * 128:(j + 1) * 128],
                                qT[:, h, q0:q0 + CHUNK],
                                start=True, stop=True,
                                skip_group_check=True)
                        drain()
                        pjt = pjp.tile([128, 2, CHUNK], bf16, tag="pj",
                                       name=f"pj_{g}_{hp}_{p}_{i}")
                        nc.scalar.activation(out=pjt[:], in_=st2[:],
                                             func=exp_, bias=ebias[:],
                                             scale=1.0)
                        if i == 0:
                            padq[p] = padp.tile([128, 2, CHUNK], f8,
                                                tag="padq",
                                                name=f"pq_{g}_{hp}_{p}")
                        nc.vector.tensor_add(padq[p][:, i, :],
                                             pjt[:, 0, :], pjt[:, 1, :])
                        if i == 1:
                            pend_sums.append(("dr", 0, padq[p], 0))
                        pend_pv.append((2 * p, pjt, i, (0, 0)))
                        fill()
                # diagonal pairs: j = 4g+2dp, 4g+2dp+1
                for dp in range(2):
                    for i, h in enumerate(heads):
                        j0 = 4 * g + 2 * dp
                        c00, c01 = 256 * dp, 256 * dp + 128
                        st2 = pst2.tile([128, 2, CHUNK], f32, tag="pst2",
                                        name=f"std_{g}_{hp}_{dp}_{i}")
                        nc.tensor.matmul(st2[:, 0, c00:],
                                         kT[:, j0 * 128:(j0 + 1) * 128],
                                         qT[:, h, q0 + c00:q0 + CHUNK],
                                         start=True, stop=True,
                                         skip_group_check=True)
                        nc.tensor.matmul(st2[:, 1, c01:],
                                         kT[:, (j0 + 1) * 128:(j0 + 2) * 128],
                                         qT[:, h, q0 + c01:q0 + CHUNK],
                                         start=True, stop=True,
                                         skip_group_check=True)
                        drain()
                        pjt = pjp.tile([128, 2, CHUNK], bf16, tag="pj",
                                       name=f"pjd_{g}_{hp}_{dp}_{i}")
                        nc.scalar.activation(out=pjt[:, 0, c00:],
                                             in_=st2[:, 0, c00:],
                                             func=exp_, bias=ebias[:],
                                             scale=1.0)
                        nc.scalar.activation(out=pjt[:, 1, c01:],
                                             in_=st2[:, 1, c01:],
                                             func=exp_, bias=ebias[:],
                                             scale=1.0)
                        nc.vector.memset(pjt[:, 1, c00:c01], 0.0)
                        for jj, cc in ((0, c00), (1, c01)):
                            nc.gpsimd.affine_select(
                                out=pjt[:, jj, cc:cc + 128],
                                in_=pjt[:, jj, cc:cc + 128],
                                pattern=[[1, 128]],
                                compare_op=mybir.AluOpType.is_ge,
                                fill=0.0,
                                base=0,
                                channel_multiplier=-1)
                        pdd = paddp.tile([128, CHUNK], bf16, tag="padd",
                                         name=f"pd_{g}_{hp}_{dp}_{i}")
                        nc.vector.tensor_add(pdd[:, c00:], pjt[:, 0, c00:],
                                             pjt[:, 1, c00:])
                        pend_sums.append(("last" if dp == 1 else "bf",
                                          i, pdd, c00))
                        pend_pv.append((j0, pjt, i, (c00, c01)))
                        fill()
                drain(0, 0)
                # finalize: r64 = 4/denominator; y8/dy8 split for oproj
                for i, h in enumerate(heads):
                    r64 = atmp.tile([64, CHUNK], f32, tag="r64",
                                    name=f"r_{g}_{hp}_{i}")
                    nc.vector.reciprocal(r64[:], sms[i])
                    t = ttp.tile([128, CHUNK], f32, tag="t",
                                 name=f"t_{g}_{hp}_{i}")
                    for half in range(2):
                        p0 = 64 * half
                        tsl = t[p0:p0 + 64, :]
                        nc.vector.tensor_mul(tsl, ys[i][p0:p0 + 64, :],
                                             r64[:])
                        y8sl = y8t[p0:p0 + 64, h, :]
                        if g == 3:
                            # tail: DVE is idle and faster than the Q7 chain
                            nc.vector.tensor_copy(out=y8sl, in_=tsl)
                            nc.vector.tensor_sub(dy8t[p0:p0 + 64, h, :],
                                                 tsl, y8sl)
                        else:
                            nc.gpsimd.tensor_copy(out=y8sl, in_=tsl)
                            nc.gpsimd.tensor_sub(dy8t[p0:p0 + 64, h, :],
                                                 tsl, y8sl)
                    fill()
                while fi[0] < len(fillers):
                    fillers[fi[0]]()
                    fi[0] += 1

            def oproj_units(g, alt=False, split_store=False):
                """16 oproj unit closures + final store; 3-term fp8
                DoubleRow over head pairs. alt: alternate pox/py pools
                (tail group, attention pools free)."""
                obuf = obp.tile([128, 4, D], bf16, tag="obuf",
                                name=f"ob_{g}")
                y8t, dy8t = y8s[g]
                units = []

                def unit(tb, oc, n):
                    def go():
                        pool = py if (alt and n % 2 == 1) else pox
                        po = pool.tile([128, CHUNK], f32,
                                       tag="py" if (alt and n % 2 == 1)
                                       else "pox",
                                       name=f"op_{g}_{tb}_{oc}")
                        first, last = True, None
                        mms = []
                        for hp in range(2):
                            h0 = 2 * hp
                            for term in range(3):
                                ysl = dy8t if term == 1 else y8t
                                w_i = 1 if term == 2 else 0
                                mms.append((
                                    ysl[:, h0:h0 + 2,
                                        tb * 128:(tb + 1) * 128],
                                    ow_s[:, w_i, h0:h0 + 2,
                                         oc * CHUNK:(oc + 1) * CHUNK]))
                        for mi, (lhsT, rhs) in enumerate(mms):
                            nc.tensor.matmul(po[:], lhsT, rhs,
                                             start=(mi == 0),
                                             stop=(mi == len(mms) - 1),
                                             perf_mode=DR,
                                             skip_group_check=True)
                        dst = obuf[:, tb, oc * CHUNK:(oc + 1) * CHUNK]
                        if n % 2 == 0:
                            nc.scalar.activation(out=dst, in_=po[:],
                                                 func=copy_,
                                                 scale=1.0 / 128.0)
                        else:
                            nc.vector.tensor_scalar_mul(dst, po[:],
                                                        1.0 / 128.0)
                        if split_store and oc == 3:
                            nc.sync.dma_start(outd[g, :, tb, :],
                                              obuf[:, tb, :])
                    return go

                n = 0
                for tb in range(4):
                    for oc in range(4):
                        units.append(unit(tb, oc, n))
                        n += 1
                if not split_store:
                    def store():
                        nc.sync.dma_start(outd[g], obuf[:])
                    units.append(store)
                return units

            # ---- schedule ----
            f0 = proj_chunk(0, first=True)
            f1 = proj_chunk(1, extra=wdma_tables)
            f0()
            f2 = proj_chunk(2, extra=wdma_ow)
            f1()
            attn_pass(0, 0)
            f3 = proj_chunk(3)
            attn_pass(0, 1)
            attn_pass(1, 0, fillers=oproj_units(0))
            f2()
            attn_pass(1, 1)
            f3()
            attn_pass(2, 0, fillers=oproj_units(1))
            attn_pass(2, 1)
            attn_pass(3, 0, fillers=oproj_units(2))
            attn_pass(3, 1)
            for u in oproj_units(3, alt=True, split_store=True):
                u()

    nc.compile()
    return nc


_CACHED = {}
LAST_EXEC_NS = None


def _run(nc, in_maps, **kwargs):
    from concourse.bass_utils import run_bass_kernel_spmd
    return run_bass_kernel_spmd(nc, in_maps, core_ids=list(range(NCORES)),
                                **kwargs)


def _make_in_maps(x, qw, kw, vw, ow, qg):
    import ml_dtypes
    bf = ml_dtypes.bfloat16
    f8 = ml_dtypes.float8_e4m3

    def split8(a, scale):
        a32 = np.asarray(a, np.float32) * np.float32(scale)
        a8 = a32.astype(f8)
        da = (a32 - a8.astype(np.float32)).astype(f8)
        return np.stack([a8, da])  # [2, ...]

    cosT, sinT = _rope_tables()
    cossin = np.concatenate([cosT, sinT], axis=0).astype(bf)   # [128, T]
    sincos = np.concatenate([sinT, cosT], axis=0).astype(bf)   # [128, T]

    def xlayout(xb):
        s = split8(xb.reshape(T, D).T, 1.0)        # [2, D, T]
        return np.ascontiguousarray(
            s.reshape(2, 4, 4, 128, T).transpose(1, 0, 2, 3, 4))

    xcats = [xlayout(x[b]) for b in range(B)]

    in_maps = []
    for c in range(NCORES):
        b, kvg = c // 4, c % 4
        h0 = HPC * kvg
        def pmajor(a, m):
            # [2, D_or_512, m] -> [128, 2, ko, m] partition-major
            return np.ascontiguousarray(
                a.reshape(2, -1, 128, m).transpose(2, 0, 1, 3))

        qwcat = pmajor(split8(qw[h0 * HD:(h0 + HPC) * HD, :].T, WSCALE), 512)
        kwcat = pmajor(split8(kw[kvg * HD:(kvg + 1) * HD, :].T, WSCALE), HD)
        vwcat = pmajor(split8(vw[kvg * HD:(kvg + 1) * HD, :].T, WSCALE), HD)
        owcat = pmajor(split8(ow[:, h0 * HD:(h0 + HPC) * HD].T, WSCALE), D)
        # s_i folds qg gain and 1/sqrt(HD); 32x psum scale cancels in the
        # ratio, only the EPS bias needs the 32^2 factor.
        s = np.array([qg[h0] / np.sqrt(HD), qg[h0 + 1] / np.sqrt(HD),
                      qg[h0 + 2] / np.sqrt(HD), qg[h0 + 3] / np.sqrt(HD),
                      1.0], np.float32)
        normo = np.broadcast_to(
            (1.0 / (HD * s * s))[None, :, None], (128, 5, 128)
        ).astype(np.float32).copy()
        normb = np.broadcast_to(
            (EPS * WSCALE * WSCALE / (s * s))[None, :],
            (128, 5)).astype(np.float32).copy()
        in_maps.append({
            "xcat": xcats[b],
            "qwcat": np.ascontiguousarray(qwcat),
            "kwcat": np.ascontiguousarray(kwcat),
            "vwcat": np.ascontiguousarray(vwcat),
            "owcat": np.ascontiguousarray(owcat),
            "csd": cossin, "csd2": sincos,
            "normod": normo, "normbd": normb,
        })
    return in_maps


def kernel(x, qw, kw, vw, ow, qg):
    global LAST_EXEC_NS
    x = np.ascontiguousarray(x, dtype=np.float32)
    qw = np.asarray(qw, dtype=np.float32)
    kw = np.asarray(kw, dtype=np.float32)
    vw = np.asarray(vw, dtype=np.float32)
    ow = np.asarray(ow, dtype=np.float32)
    qg = np.asarray(qg, dtype=np.float32)

    if "nc" not in _CACHED:
        _CACHED["nc"] = _build_program()
    nc = _CACHED["nc"]

    in_maps = _make_in_maps(x, qw, kw, vw, ow, qg)
    res = _run(nc, in_maps)
    LAST_EXEC_NS = res.exec_time_ns
    out = np.empty((B, T, D), np.float32)
    for b in range(B):
        acc = np.zeros((4, 128, 4, D), np.float32)
        for kvg in range(4):
            acc += res.results[4 * b + kvg]["o"].astype(np.float32)
        # [g, p, tb, d] -> rows g*512 + tb*128 + p
        out[b] = acc.transpose(0, 2, 1, 3).reshape(T, D)
    return np.ascontiguousarray(out)


# revision 25
# speedup vs baseline: 1.2704x; 1.0390x over previous
"""Bass/Tile kernel for nn_Attn_40424232189956 on 8 trn2 NeuronCores.

GQA attention block: q/k/v proj + rmsnorm + rope + causal attention + out proj.
B=2, T=2048, D=2048, NH=16, NKV=4, HD=128.

Sharding: core c -> (batch b = c//4, kv-group kvg = c%4). Each core owns one
batch's tokens, q heads 4*kvg..4*kvg+3 and kv head kvg; it computes a full
[T, D] partial of the output projection and the host sums the 4 partials per
batch. Unlike the head-only sharding this removes all duplicated k/v
projection work and halves both the x load and the output store traffic.

Performance structure (tuned against the TimelineSim cost model):
- Projections and the output projection run as THREE fp8 (e4m3) DoubleRow
  matmul terms: a8@b8 + da8@b8 + a8@db8, where da/db are the fp8 residuals
  of the bf16-class value. DoubleRow processes two 128-contraction tiles per
  instruction at 0.5 cycles/moving-column in the cost model, so the 3-term
  split costs 0.75x of the bf16 pair while keeping ~bf16 accuracy (the
  dropped da@db term and residual quantization are ~0.1% effects).
  Weights ship 32-scaled (fp8 subnormal cutoff), x unscaled; the rmsnorm
  absorbs the 32x PSUM scale for q/k (EPS bias scaled by 1024), the v copy
  divides by 32, and the oproj staging copy divides by 128 (=4*32, the 4
  from folding 0.25 into the softmax-denominator ones matmul).
- Attention (scores, exp, pv) stays bf16: fp8 there fails the 2e-2 gate.
  Far (fully-causal) key-block pairs share one [128,2,512] PSUM stile and a
  single [128,1024] exp; diagonal blocks keep windowed exps + gpsimd
  affine_select masking. exp carries a -2 bias (cancelled by the softmax
  normalization) so pj and the fp8 pads stay inside e4m3 range.
- Softmax denominators: far pad pairs are fp8 and summed with a DoubleRow
  ones matmul (4x cheaper); diagonal pads stay bf16 ones-matmuls so the
  short early rows keep bf16-accurate denominators.
- Normalized attention output is split y8+dy8 (fp8 + fp8 residual) on
  DVE/gpsimd for the 3-term oproj; r64 multiplies by 4/denominator.
- oproj units (16 per group) are interleaved into the NEXT group's first
  attention pass so the PE never waits on the normalize chain; the last
  group alternates two PSUM pools and stores per-128-token-block.
- Engine balance: ACT = qsb/v copies, Square, Sqrt, exp, half the staging
  copies (sqrt and exp eras separated to minimize act-table loads); DVE =
  rope mixes (bf16 2x), qn, reciprocals, pad adds, t-muls, other staging
  copies; gpsimd = one rope mix, affine_select, y8 quantize + dy residual.
"""

import numpy as np

B, T, D = 2, 2048, 2048
NH, NKV = 16, 4
HD = 128
NCORES = 8
HPC = 4               # q heads per core
NKT = D // 128        # 16 contraction tiles for projections
CHUNK = 512
EPS = float(np.finfo(np.float32).eps)
EXP_BIAS = -2.0
WSCALE = 32.0


def _rope_tables():
    # Matches reference.rotary_tables for T=2048 > tsl=1024 (NTK branch).
    hd = np.float32(HD)
    ar = (np.arange(0, HD, 2, dtype=np.float32) / hd).astype(np.float32)
    expo = np.power(np.float32(HD / (HD - 2.0)), ar, dtype=np.float32)
    inv = (np.float32(1.0)
           / (np.float32(10000.0)
              * np.power(np.float32(T / 1024.0), expo, dtype=np.float32)))
    f = np.outer(np.arange(T, dtype=np.float32), inv.astype(np.float32))
    return (np.cos(f).astype(np.float32).T.copy(),
            np.sin(f).astype(np.float32).T.copy())  # [64, T] hd-major


def _build_program():
    import concourse.bass as bass
    import concourse.mybir as mybir
    import concourse.tile as tile
    from concourse import bacc
    import concourse.hw_specs as hw_specs

    # The act-table-load inserter picks the FIRST table set containing each
    # activation func, so a kernel mixing Ln (rmsnorm rsqrt) and Exp
    # (softmax) flip-flops between the 'natural_log' and 'exp_and_others'
    # sets, paying ~1.28us per switch. Every func we use (Ln, Exp, Square,
    # Copy) lives together in 'natural_log_exp_and_others'; empty the other
    # sets (keeping dict order, so canonical act_func_set_ids are unchanged
    # for walrus) to steer the chooser there once.
    if not getattr(bacc.get_activation_tables, "_attn_patched", False):
        _orig_gat = bacc.get_activation_tables

        def _gat(arch):
            t = _orig_gat(arch)
            keep = "natural_log_exp_and_others"
            if keep not in t:
                return t
            return {k: (v if k == keep else frozenset())
                    for k, v in t.items()}

        _gat._attn_patched = True
        bacc.get_activation_tables = _gat
        hw_specs_gat = getattr(hw_specs, "get_activation_tables", None)
        if hw_specs_gat is not None and not getattr(
                hw_specs_gat, "_attn_patched", False):
            hw_specs.get_activation_tables = _gat

    f32 = mybir.dt.float32
    f32r = mybir.dt.float32r
    bf16 = mybir.dt.bfloat16
    f8 = mybir.dt.float8e4
    DR = mybir.MatmulPerfMode.DoubleRow
    nc = bacc.Bacc("TRN2", target_bir_lowering=False)

    # x layout [kg, base/resid, ko-within-group, p, t] so (base/resid, ko)
    # merge into one 3D-balanceable access pattern per chunk DMA.
    xcat = nc.dram_tensor("xcat", [4, 2, 4, 128, T], f8,
                          kind="ExternalInput")
    # weights ship partition-major so each DMA is one contiguous run per
    # partition (512B+ descriptors: full DMA bus bandwidth in the model)
    qwcat = nc.dram_tensor("qwcat", [128, 2, NKT, HPC * HD], f8,
                           kind="ExternalInput")
    kwcat = nc.dram_tensor("kwcat", [128, 2, NKT, HD], f8,
                           kind="ExternalInput")
    vwcat = nc.dram_tensor("vwcat", [128, 2, NKT, HD], f8,
                           kind="ExternalInput")
    owcat = nc.dram_tensor("owcat", [128, 2, HPC, D], f8,
                           kind="ExternalInput")
    csd = nc.dram_tensor("csd", [128, T], bf16, kind="ExternalInput")
    csd2 = nc.dram_tensor("csd2", [128, T], bf16, kind="ExternalInput")
    normod = nc.dram_tensor("normod", [128, 5, 128], f32r, kind="ExternalInput")
    normbd = nc.dram_tensor("normbd", [128, 5], f32, kind="ExternalInput")
    # [g, p, tb, d]; host reassembles rows g*512 + tb*128 + p.
    outd = nc.dram_tensor("o", [4, 128, 4, D], bf16, kind="ExternalOutput")

    xr = xcat.rearrange("kg two ko p t -> p kg two ko t")

    sq_ = mybir.ActivationFunctionType.Square
    ln_ = mybir.ActivationFunctionType.Ln
    exp_ = mybir.ActivationFunctionType.Exp
    copy_ = mybir.ActivationFunctionType.Copy

    with tile.TileContext(nc) as tc:
        with (
            tc.tile_pool(name="wpool", bufs=1) as wpool,
            tc.tile_pool(name="xpool", bufs=2) as xpool,
            tc.tile_pool(name="big", bufs=1) as big,
            tc.tile_pool(name="vtp", bufs=2) as vtp,
            tc.tile_pool(name="ntmp", bufs=2) as ntmp,
            tc.tile_pool(name="ntm2", bufs=1) as ntm2,
            tc.tile_pool(name="pjp", bufs=6) as pjp,
            tc.tile_pool(name="padp", bufs=3) as padp,
            tc.tile_pool(name="padd", bufs=3) as paddp,
            tc.tile_pool(name="ttp", bufs=2) as ttp,
            tc.tile_pool(name="y8p", bufs=2) as y8p,
            tc.tile_pool(name="dy8p", bufs=2) as dy8p,
            tc.tile_pool(name="atmp", bufs=2) as atmp,
            tc.tile_pool(name="obp", bufs=1) as obp,
            tc.tile_pool(name="pst2", bufs=2, space="PSUM") as pst2,
            tc.tile_pool(name="py", bufs=2, space="PSUM") as py,
            tc.tile_pool(name="psm", bufs=1, space="PSUM") as psm,
            tc.tile_pool(name="pox", bufs=1, space="PSUM") as pox,
        ):
            # ---- resident weights / tables (DMAs emitted lazily below) ----
            qw_s = wpool.tile([128, 2, NKT, HPC * HD], f8)
            kw_s = wpool.tile([128, 2, NKT, HD], f8)
            vw_s = wpool.tile([128, 2, NKT, HD], f8)
            ow_s = wpool.tile([128, 2, HPC, D], f8)
            cs_s = wpool.tile([128, T], bf16)   # rows 0:64 cos, 64:128 sin
            cs2_s = wpool.tile([128, T], bf16)  # rows 0:64 sin, 64:128 cos
            normo_s = wpool.tile([128, 5, 128], f32r)
            normb_s = wpool.tile([128, 5], f32)
            ones_bf = wpool.tile([128, 64], bf16)
            nc.vector.memset(ones_bf[:], 0.25)
            # masked DR ones: rhs slot 0 (head 0's pad) -> rows 0:64, slot 1
            # (head 1's pad) -> rows 64:128, one DoubleRow matmul for both.
            ones8 = wpool.tile([128, 2, 128], f8)
            nc.vector.memset(ones8[:], 0.0)
            nc.vector.memset(ones8[:, 0, 0:64], 0.25)
            nc.vector.memset(ones8[:, 1, 64:128], 0.25)
            ebias = wpool.tile([128, 1], f32)
            nc.vector.memset(ebias[:], EXP_BIAS)
            zbias = wpool.tile([128, 1], f32)
            nc.vector.memset(zbias[:], 0.0)

            qT = big.tile([128, HPC, T], bf16)
            kT = big.tile([128, T], bf16)
            vtok = big.tile([128, T], bf16)

            def wdma_c0():
                # after chunk0's x: tables (eager norms need them early),
                # then the qw residual and the k/v base weights
                nc.sync.dma_start(normo_s[:], normod[:])
                nc.sync.dma_start(normb_s[:], normbd[:])
                nc.sync.dma_start(cs_s[:], csd[:])
                nc.sync.dma_start(cs2_s[:], csd2[:])
                nc.sync.dma_start(qw_s[:, 1, 0:8], qwcat[:, 1, 0:8])
                nc.sync.dma_start(qw_s[:, 1, 8:16], qwcat[:, 1, 8:16])
                nc.sync.dma_start(kw_s[:, 0], kwcat[:, 0])
                nc.sync.dma_start(vw_s[:, 0], vwcat[:, 0])
                nc.sync.dma_start(kw_s[:, 1], kwcat[:, 1])
                nc.sync.dma_start(vw_s[:, 1], vwcat[:, 1])

            def wdma_ow():
                nc.sync.dma_start(ow_s[:, 0], owcat[:, 0])
                nc.sync.dma_start(ow_s[:, 1], owcat[:, 1])

            def norm_math(qsb, ni, dst, pos0):
                """qsb: PSUM f32 [128 feat, 512 tok] at 32x scale; ni: 0..3 q
                heads, 4 k. dst: [128, 512] bf16 slice of qT/kT. rmsnorm
                (qg & attn scale folded, 32x absorbed) + rope, hd-major.
                Reads the projection PSUM directly (no SBUF staging copy)."""
                sq = ntmp.tile([128, CHUNK], f32r, tag="sq")
                nc.scalar.activation(out=sq[:], in_=qsb[:], func=sq_)
                nb = pox.tile([128, CHUNK], f32, tag="pox",
                              name=f"nb_{ni}_{pos0}")
                nc.tensor.matmul(nb[:], normo_s[:, ni, :], sq[:],
                                 start=True, stop=True)
                # rfac = rsqrt(nb + eps) = exp(-0.5*ln(.)): keeps every ACT
                # func in the natural_log_exp table set (no table reloads)
                # and needs no DVE reciprocal.
                rs = ntmp.tile([128, CHUNK], f32, tag="rs")
                nc.scalar.activation(out=rs[:], in_=nb[:], func=ln_,
                                     bias=normb_s[:, ni:ni + 1], scale=1.0)
                rfac = ntmp.tile([128, CHUNK], f32, tag="rfac")
                nc.scalar.activation(out=rfac[:], in_=rs[:], func=exp_,
                                     bias=zbias[:], scale=-0.5)
                qn = ntmp.tile([128, CHUNK], bf16, tag="qn")
                nc.vector.tensor_mul(qn[:], qsb[:], rfac[:])
                cs = cs_s[0:64, pos0:pos0 + CHUNK]       # cos @ base 0
                sn = cs_s[64:128, pos0:pos0 + CHUNK]     # sin @ base 64
                sn0 = cs2_s[0:64, pos0:pos0 + CHUNK]     # sin @ base 0
                cs64 = cs2_s[64:128, pos0:pos0 + CHUNK]  # cos @ base 64
                t1 = ntm2.tile([64, CHUNK], bf16, tag="ta")
                t2 = ntm2.tile([64, CHUNK], bf16, tag="tb")
                nc.gpsimd.tensor_mul(t1[:], qn[0:64, :], cs)
                nc.vector.tensor_mul(t2[:], qn[64:128, :], sn)
                nc.vector.tensor_add(dst[0:64, :], t1[:], t2[:])
                t3 = ntm2.tile([64, CHUNK], bf16, tag="tc")
                t4 = ntm2.tile([64, CHUNK], bf16, tag="tb")
                nc.vector.tensor_mul(t3[:], qn[0:64, :], sn0)
                nc.vector.tensor_mul(t4[:], qn[64:128, :], cs64)
                nc.vector.tensor_sub(dst[64:128, :], t4[:], t3[:])

            def emit_xdmas(ci, first=False, extra=None):
                t0 = ci * CHUNK
                xt = xpool.tile([128, 4, 2, 4, CHUNK], f8, tag="xt",
                                name=f"xt_{ci}")
                if first:
                    # split by kg so the first matmuls start early, with the
                    # weight DMAs spliced between
                    qwb = [lambda: nc.sync.dma_start(qw_s[:, 0, 0:1],
                                                     qwcat[:, 0, 0:1]),
                           lambda: nc.sync.dma_start(qw_s[:, 0, 1:8],
                                                     qwcat[:, 0, 1:8]),
                           lambda: nc.sync.dma_start(qw_s[:, 0, 8:16],
                                                     qwcat[:, 0, 8:16])]
                    for kg in range(4):
                        nc.sync.dma_start(
                            xt[:, kg], xr[:, kg, :, :, t0:t0 + CHUNK])
                        if kg < 3:
                            qwb[kg]()
                else:
                    nc.sync.dma_start(xt[:], xr[:, :, :, :, t0:t0 + CHUNK])
                if extra is not None:
                    extra()
                return xt

            def proj_pass(xt, w_s, blks, pos0, tagn):
                """One 2-output-block pass of the 3-term fp8 projection.
                blks: two col0 stationary selectors into w_s's last dim.
                Returns the [128,2,CHUNK] psum tile."""
                pq = pst2.tile([128, 2, CHUNK], f32, tag="pst2",
                               name=f"pq_{tagn}_{pos0}")
                for term in range(3):
                    xt_i = 1 if term == 1 else 0
                    w_i = 1 if term == 2 else 0
                    for pr in range(8):
                        rhs = xt[:, pr // 2, xt_i,
                                 2 * (pr % 2):2 * (pr % 2) + 2, :]
                        st = (term == 0 and pr == 0)
                        sp = (term == 2 and pr == 7)
                        for i, c0 in enumerate(blks):
                            lhsT = w_s[:, w_i, 2 * pr:2 * pr + 2,
                                       c0:c0 + 128]
                            nc.tensor.matmul(pq[:, i, :], lhsT, rhs,
                                             start=st, stop=sp,
                                             perf_mode=DR,
                                             skip_group_check=True)
                return pq

            def proj_chunk(ci, first=False, extra=None):
                pos0 = ci * CHUNK
                xt = emit_xdmas(ci, first=first, extra=extra)
                pq = proj_pass(xt, qw_s, (0, 128), pos0, "q01")
                for i in range(2):
                    norm_math(pq[:, i, :], i, qT[:, i, pos0:pos0 + CHUNK],
                              pos0)
                pq = proj_pass(xt, qw_s, (256, 384), pos0, "q23")
                for i in range(2):
                    norm_math(pq[:, i, :], i + 2,
                              qT[:, i + 2, pos0:pos0 + CHUNK], pos0)
                pkv = pst2.tile([128, 2, CHUNK], f32, tag="pst2",
                                name=f"pkv_{ci}")
                for term in range(3):
                    xt_i = 1 if term == 1 else 0
                    w_i = 1 if term == 2 else 0
                    for pr in range(8):
                        rhs = xt[:, pr // 2, xt_i,
                                 2 * (pr % 2):2 * (pr % 2) + 2, :]
                        st = (term == 0 and pr == 0)
                        sp = (term == 2 and pr == 7)
                        nc.tensor.matmul(pkv[:, 0, :],
                                         kw_s[:, w_i, 2 * pr:2 * pr + 2, :],
                                         rhs, start=st, stop=sp,
                                         perf_mode=DR, skip_group_check=True)
                        nc.tensor.matmul(pkv[:, 1, :],
                                         vw_s[:, w_i, 2 * pr:2 * pr + 2, :],
                                         rhs, start=st, stop=sp,
                                         perf_mode=DR, skip_group_check=True)
                vtmp = vtp.tile([128, CHUNK], bf16, tag="vtmp",
                                name=f"vtmp_{ci}")
                nc.scalar.activation(out=vtmp[:], in_=pkv[:, 1, :],
                                     func=copy_, scale=1.0 / WSCALE)
                norm_math(pkv[:, 0, :], 4, kT[:, pos0:pos0 + CHUNK], pos0)
                for tb in range(4):
                    dst0 = pos0 + tb * 128
                    nc.sync.dma_start_transpose(
                        vtok[:, dst0:dst0 + 128],
                        vtmp[:, tb * 128:(tb + 1) * 128])

            y8s = {}

            def attn_pass(g, hp, fillers=()):
                """Scores/exp/pads/pv/sums for heads (2hp, 2hp+1) of query
                group g; finalize writes y8/dy8. fillers: emission closures
                (previous group's oproj units) spread across the pass."""
                kg = 4 * (g + 1)
                q0 = g * CHUNK
                heads = (2 * hp, 2 * hp + 1)
                if hp == 0:
                    y8s[g] = (y8p.tile([128, HPC, CHUNK], f8, tag="y8",
                                       name=f"y8_{g}"),
                              dy8p.tile([128, HPC, CHUNK], f8, tag="dy8",
                                        name=f"dy8_{g}"))
                y8t, dy8t = y8s[g]
                ys = {}
                for i, h in enumerate(heads):
                    ys[i] = py.tile([128, CHUNK], f32, tag="py",
                                    name=f"y_{g}_{hp}_{i}")
                smt = psm.tile([128, CHUNK], f32, tag="psm",
                               name=f"sm_{g}_{hp}")
                sms = [smt[0:64, :], smt[64:128, :]]

                n_far = 2 * g            # far j-pairs per head
                stepn = [0]
                n_steps = (n_far + 2) * 2 + 2
                fi = [0]

                def fill():
                    stepn[0] += 1
                    want = min(len(fillers),
                               (stepn[0] * len(fillers)) // n_steps + 1)
                    while fi[0] < want:
                        fillers[fi[0]]()
                        fi[0] += 1

                pend_pv = []
                pend_sums = []
                sums_started = [False, False]

                def emit_pv(unit):
                    jj0, pjt, i, c0s = unit
                    for jj in range(2):
                        j = jj0 + jj
                        nc.tensor.matmul(
                            ys[i][:, c0s[jj]:],
                            vtok[:, j * 128:(j + 1) * 128],
                            pjt[:, jj, c0s[jj]:],
                            start=(j == 0), stop=(j == kg - 1),
                            skip_group_check=True)

                def emit_sums(unit):
                    kind, i, op, c0 = unit
                    if kind == "dr":
                        # writes both heads' 64-row ranges at once
                        st = not (sums_started[0] or sums_started[1])
                        sums_started[0] = sums_started[1] = True
                        nc.tensor.matmul(smt[:, c0:], ones8[:],
                                         op[:, :, c0:], start=st, stop=False,
                                         perf_mode=DR, skip_group_check=True)
                    else:
                        st = not sums_started[i]
                        sums_started[i] = True
                        nc.tensor.matmul(sms[i][:, c0:], ones_bf[:],
                                         op[:, c0:], start=st,
                                         stop=(kind == "last"),
                                         skip_group_check=True)

                def drain(pv_keep=1, sums_keep=1):
                    while len(pend_pv) > pv_keep:
                        emit_pv(pend_pv.pop(0))
                    while len(pend_sums) > sums_keep:
                        emit_sums(pend_sums.pop(0))

                padq = {}
                # far pairs: j = 2p, 2p+1; fully causal, full query width
                for p in range(n_far):
                    for i, h in enumerate(heads):
                        st2 = pst2.tile([128, 2, CHUNK], f32, tag="pst2",
                                        name=f"st_{g}_{hp}_{p}_{i}")
                        for jj in range(2):
                            j = 2 * p + jj
                            nc.tensor.matmul(
                                st2[:, jj, :],
                                kT[:, j * 128:(j + 1) * 128],
                                qT[:, h, q0:q0 + CHUNK],
                                start=True, stop=True,
                                skip_group_check=True)
                        drain()
                        pjt = pjp.tile([128, 2, CHUNK], bf16, tag="pj",
                                       name=f"pj_{g}_{hp}_{p}_{i}")
                        nc.scalar.activation(out=pjt[:], in_=st2[:],
                                             func=exp_, bias=ebias[:],
                                             scale=1.0)
                        if i == 0:
                            padq[p] = padp.tile([128, 2, CHUNK], f8,
                                                tag="padq",
                                                name=f"pq_{g}_{hp}_{p}")
                        nc.vector.tensor_add(padq[p][:, i, :],
                                             pjt[:, 0, :], pjt[:, 1, :])
                        if i == 1:
                            pend_sums.append(("dr", 0, padq[p], 0))
                        pend_pv.append((2 * p, pjt, i, (0, 0)))
                        fill()
                # diagonal pairs: j = 4g+2dp, 4g+2dp+1
                for dp in range(2):
                    for i, h in enumerate(heads):
                        j0 = 4 * g + 2 * dp
                        c00, c01 = 256 * dp, 256 * dp + 128
                        st2 = pst2.tile([128, 2, CHUNK], f32, tag="pst2",
                                        name=f"std_{g}_{hp}_{dp}_{i}")
                        nc.tensor.matmul(st2[:, 0, c00:],
                                         kT[:, j0 * 128:(j0 + 1) * 128],
                                         qT[:, h, q0 + c00:q0 + CHUNK],
                                         start=True, stop=True,
                                         skip_group_check=True)
                        nc.tensor.matmul(st2[:, 1, c01:],
                                         kT[:, (j0 + 1) * 128:(j0 + 2) * 128],
                                         qT[:, h, q0 + c01:q0 + CHUNK],
                                         start=True, stop=True,
                                         skip_group_check=True)
                        drain()
                        pjt = pjp.tile([128, 2, CHUNK], bf16, tag="pj",
                                       name=f"pjd_{g}_{hp}_{dp}_{i}")
                        nc.scalar.activation(out=pjt[:, 0, c00:],
                                             in_=st2[:, 0, c00:],
                                             func=exp_, bias=ebias[:],
                                             scale=1.0)
                        nc.scalar.activation(out=pjt[:, 1, c01:],
                                             in_=st2[:, 1, c01:],
                                             func=exp_, bias=ebias[:],
                                             scale=1.0)
                        nc.vector.memset(pjt[:, 1, c00:c01], 0.0)
                        for jj, cc in ((0, c00), (1, c01)):
                            nc.gpsimd.affine_select(
                                out=pjt[:, jj, cc:cc + 128],
                                in_=pjt[:, jj, cc:cc + 128],
                                pattern=[[1, 128]],
                                compare_op=mybir.AluOpType.is_ge,
                                fill=0.0,
                                base=0,
                                channel_multiplier=-1)
                        pdd = paddp.tile([128, CHUNK], bf16, tag="padd",
                                         name=f"pd_{g}_{hp}_{dp}_{i}")
                        nc.vector.tensor_add(pdd[:, c00:], pjt[:, 0, c00:],
                                             pjt[:, 1, c00:])
                        pend_sums.append(("last" if dp == 1 else "bf",
                                          i, pdd, c00))
                        pend_pv.append((j0, pjt, i, (c00, c01)))
                        fill()
                drain(0, 0)
                # finalize: r64 = 4/denominator; y8/dy8 split for oproj
                for i, h in enumerate(heads):
                    r64 = atmp.tile([64, CHUNK], f32, tag="r64",
                                    name=f"r_{g}_{hp}_{i}")
                    nc.vector.reciprocal(r64[:], sms[i])
                    t = ttp.tile([128, CHUNK], f32, tag="t",
                                 name=f"t_{g}_{hp}_{i}")
                    for half in range(2):
                        p0 = 64 * half
                        tsl = t[p0:p0 + 64, :]
                        nc.vector.tensor_mul(tsl, ys[i][p0:p0 + 64, :],
                                             r64[:])
                        y8sl = y8t[p0:p0 + 64, h, :]
                        if g == 3:
                            # tail: DVE is idle and faster than the Q7 chain
                            nc.vector.tensor_copy(out=y8sl, in_=tsl)
                            nc.vector.tensor_sub(dy8t[p0:p0 + 64, h, :],
                                                 tsl, y8sl)
                        else:
                            nc.gpsimd.tensor_copy(out=y8sl, in_=tsl)
                            nc.gpsimd.tensor_sub(dy8t[p0:p0 + 64, h, :],
                                                 tsl, y8sl)
                    fill()
                while fi[0] < len(fillers):
                    fillers[fi[0]]()
                    fi[0] += 1

            def oproj_units(g, alt=False, split_store=False):
                """16 oproj unit closures + final store; 3-term fp8
                DoubleRow over head pairs. alt: alternate pox/py pools
                (tail group, attention pools free)."""
                obuf = obp.tile([128, 4, D], bf16, tag="obuf",
                                name=f"ob_{g}")
                y8t, dy8t = y8s[g]
                units = []

                def unit(tb, oc, n):
                    def go():
                        pool = py if (alt and n % 2 == 1) else pox
                        po = pool.tile([128, CHUNK], f32,
                                       tag="py" if (alt and n % 2 == 1)
                                       else "pox",
                                       name=f"op_{g}_{tb}_{oc}")
                        first, last = True, None
                        mms = []
                        for hp in range(2):
                            h0 = 2 * hp
                            for term in range(3):
                                ysl = dy8t if term == 1 else y8t
                                w_i = 1 if term == 2 else 0
                                mms.append((
                                    ysl[:, h0:h0 + 2,
                                        tb * 128:(tb + 1) * 128],
                                    ow_s[:, w_i, h0:h0 + 2,
                                         oc * CHUNK:(oc + 1) * CHUNK]))
                        for mi, (lhsT, rhs) in enumerate(mms):
                            nc.tensor.matmul(po[:], lhsT, rhs,
                                             start=(mi == 0),
                                             stop=(mi == len(mms) - 1),
                                             perf_mode=DR,
                                             skip_group_check=True)
                        dst = obuf[:, tb, oc * CHUNK:(oc + 1) * CHUNK]
                        if n % 2 == 0:
                            nc.scalar.activation(out=dst, in_=po[:],
                                                 func=copy_,
                                                 scale=1.0 / 128.0)
                        else:
                            nc.vector.tensor_scalar_mul(dst, po[:],
                                                        1.0 / 128.0)
                        if split_store and oc == 3:
                            nc.sync.dma_start(outd[g, :, tb, :],
                                              obuf[:, tb, :])
                    return go

                n = 0
                for tb in range(4):
                    for oc in range(4):
                        units.append(unit(tb, oc, n))
                        n += 1
                if not split_store:
                    def store():
                        nc.sync.dma_start(outd[g], obuf[:])
                    units.append(store)
                return units

            # ---- schedule ----
            proj_chunk(0, first=True, extra=wdma_c0)
            proj_chunk(1)
            proj_chunk(2, extra=wdma_ow)
            attn_pass(0, 0)
            proj_chunk(3)
            attn_pass(0, 1)
            attn_pass(1, 0, fillers=oproj_units(0))
            attn_pass(1, 1)
            attn_pass(2, 0, fillers=oproj_units(1))
            attn_pass(2, 1)
            attn_pass(3, 0, fillers=oproj_units(2))
            attn_pass(3, 1)
            for u in oproj_units(3, alt=True, split_store=True):
                u()

    nc.compile()
    return nc


_CACHED = {}
LAST_EXEC_NS = None


def _run(nc, in_maps, **kwargs):
    from concourse.bass_utils import run_bass_kernel_spmd
    return run_bass_kernel_spmd(nc, in_maps, core_ids=list(range(NCORES)),
                                **kwargs)


def _make_in_maps(x, qw, kw, vw, ow, qg):
    import ml_dtypes
    bf = ml_dtypes.bfloat16
    f8 = ml_dtypes.float8_e4m3

    def split8(a, scale):
        a32 = np.asarray(a, np.float32) * np.float32(scale)
        a8 = a32.astype(f8)
        da = (a32 - a8.astype(np.float32)).astype(f8)
        return np.stack([a8, da])  # [2, ...]

    cosT, sinT = _rope_tables()
    cossin = np.concatenate([cosT, sinT], axis=0).astype(bf)   # [128, T]
    sincos = np.concatenate([sinT, cosT], axis=0).astype(bf)   # [128, T]

    def xlayout(xb):
        s = split8(xb.reshape(T, D).T, 1.0)        # [2, D, T]
        return np.ascontiguousarray(
            s.reshape(2, 4, 4, 128, T).transpose(1, 0, 2, 3, 4))

    xcats = [xlayout(x[b]) for b in range(B)]

    in_maps = []
    for c in range(NCORES):
        b, kvg = c // 4, c % 4
        h0 = HPC * kvg
        def pmajor(a, m):
            # [2, D_or_512, m] -> [128, 2, ko, m] partition-major
            return np.ascontiguousarray(
                a.reshape(2, -1, 128, m).transpose(2, 0, 1, 3))

        qwcat = pmajor(split8(qw[h0 * HD:(h0 + HPC) * HD, :].T, WSCALE), 512)
        kwcat = pmajor(split8(kw[kvg * HD:(kvg + 1) * HD, :].T, WSCALE), HD)
        vwcat = pmajor(split8(vw[kvg * HD:(kvg + 1) * HD, :].T, WSCALE), HD)
        owcat = pmajor(split8(ow[:, h0 * HD:(h0 + HPC) * HD].T, WSCALE), D)
        # s_i folds qg gain and 1/sqrt(HD); 32x psum scale cancels in the
        # ratio, only the EPS bias needs the 32^2 factor.
        s = np.array([qg[h0] / np.sqrt(HD), qg[h0 + 1] / np.sqrt(HD),
                      qg[h0 + 2] / np.sqrt(HD), qg[h0 + 3] / np.sqrt(HD),
                      1.0], np.float32)
        normo = np.broadcast_to(
            (1.0 / (HD * s * s))[None, :, None], (128, 5, 128)
        ).astype(np.float32).copy()
        normb = np.broadcast_to(
            (EPS * WSCALE * WSCALE / (s * s))[None, :],
            (128, 5)).astype(np.float32).copy()
        in_maps.append({
            "xcat": xcats[b],
            "qwcat": np.ascontiguousarray(qwcat),
            "kwcat": np.ascontiguousarray(kwcat),
            "vwcat": np.ascontiguousarray(vwcat),
            "owcat": np.ascontiguousarray(owcat),
            "csd": cossin, "csd2": sincos,
            "normod": normo, "normbd": normb,
        })
    return in_maps


def kernel(x, qw, kw, vw, ow, qg):
    global LAST_EXEC_NS
    x = np.ascontiguousarray(x, dtype=np.float32)
    qw = np.asarray(qw, dtype=np.float32)
    kw = np.asarray(kw, dtype=np.float32)
    vw = np.asarray(vw, dtype=np.float32)
    ow = np.asarray(ow, dtype=np.float32)
    qg = np.asarray(qg, dtype=np.float32)

    if "nc" not in _CACHED:
        _CACHED["nc"] = _build_program()
    nc = _CACHED["nc"]

    in_maps = _make_in_maps(x, qw, kw, vw, ow, qg)
    res = _run(nc, in_maps)
    LAST_EXEC_NS = res.exec_time_ns
    out = np.empty((B, T, D), np.float32)
    for b in range(B):
        acc = np.zeros((4, 128, 4, D), np.float32)
        for kvg in range(4):
            acc += res.results[4 * b + kvg]["o"].astype(np.float32)
        # [g, p, tb, d] -> rows g*512 + tb*128 + p
        out[b] = acc.transpose(0, 2, 1, 3).reshape(T, D)
    return np.ascontiguousarray(out)


# revision 27
# speedup vs baseline: 1.2937x; 1.0184x over previous
"""Bass/Tile kernel for nn_Attn_40424232189956 on 8 trn2 NeuronCores.

GQA attention block: q/k/v proj + rmsnorm + rope + causal attention + out proj.
B=2, T=2048, D=2048, NH=16, NKV=4, HD=128.

Sharding: core c -> (batch b = c//4, kv-group kvg = c%4). Each core owns one
batch's tokens, q heads 4*kvg..4*kvg+3 and kv head kvg; it computes a full
[T, D] partial of the output projection and the host sums the 4 partials per
batch. Unlike the head-only sharding this removes all duplicated k/v
projection work and halves both the x load and the output store traffic.

Performance structure (tuned against the TimelineSim cost model):
- Projections and the output projection run as THREE fp8 (e4m3) DoubleRow
  matmul terms: a8@b8 + da8@b8 + a8@db8, where da/db are the fp8 residuals
  of the bf16-class value. DoubleRow processes two 128-contraction tiles per
  instruction at 0.5 cycles/moving-column in the cost model, so the 3-term
  split costs 0.75x of the bf16 pair while keeping ~bf16 accuracy (the
  dropped da@db term and residual quantization are ~0.1% effects).
  Weights ship 32-scaled (fp8 subnormal cutoff), x unscaled; the rmsnorm
  absorbs the 32x PSUM scale for q/k (EPS bias scaled by 1024), the v copy
  divides by 32, and the oproj staging copy divides by 128 (=4*32, the 4
  from folding 0.25 into the softmax-denominator ones matmul).
- Attention (scores, exp, pv) stays bf16: fp8 there fails the 2e-2 gate.
  Far (fully-causal) key-block pairs share one [128,2,512] PSUM stile and a
  single [128,1024] exp; diagonal blocks keep windowed exps + gpsimd
  affine_select masking. exp carries a -2 bias (cancelled by the softmax
  normalization) so pj and the fp8 pads stay inside e4m3 range.
- Softmax denominators: far pad pairs are fp8 and summed with a DoubleRow
  ones matmul (4x cheaper); diagonal pads stay bf16 ones-matmuls so the
  short early rows keep bf16-accurate denominators.
- Normalized attention output is split y8+dy8 (fp8 + fp8 residual) on
  DVE/gpsimd for the 3-term oproj; r64 multiplies by 4/denominator.
- oproj units (16 per group) are interleaved into the NEXT group's first
  attention pass so the PE never waits on the normalize chain; the last
  group alternates two PSUM pools and stores per-128-token-block.
- Engine balance: ACT = qsb/v copies, Square, Sqrt, exp, half the staging
  copies (sqrt and exp eras separated to minimize act-table loads); DVE =
  rope mixes (bf16 2x), qn, reciprocals, pad adds, t-muls, other staging
  copies; gpsimd = one rope mix, affine_select, y8 quantize + dy residual.
"""

import numpy as np

B, T, D = 2, 2048, 2048
NH, NKV = 16, 4
HD = 128
NCORES = 8
HPC = 4               # q heads per core
NKT = D // 128        # 16 contraction tiles for projections
CHUNK = 512
EPS = float(np.finfo(np.float32).eps)
EXP_BIAS = -2.0
WSCALE = 32.0


def _rope_tables():
    # Matches reference.rotary_tables for T=2048 > tsl=1024 (NTK branch).
    hd = np.float32(HD)
    ar = (np.arange(0, HD, 2, dtype=np.float32) / hd).astype(np.float32)
    expo = np.power(np.float32(HD / (HD - 2.0)), ar, dtype=np.float32)
    inv = (np.float32(1.0)
           / (np.float32(10000.0)
              * np.power(np.float32(T / 1024.0), expo, dtype=np.float32)))
    f = np.outer(np.arange(T, dtype=np.float32), inv.astype(np.float32))
    return (np.cos(f).astype(np.float32).T.copy(),
            np.sin(f).astype(np.float32).T.copy())  # [64, T] hd-major


def _build_program():
    import concourse.bass as bass
    import concourse.mybir as mybir
    import concourse.tile as tile
    from concourse import bacc
    import concourse.hw_specs as hw_specs

    # The act-table-load inserter picks the FIRST table set containing each
    # activation func, so a kernel mixing Ln (rmsnorm rsqrt) and Exp
    # (softmax) flip-flops between the 'natural_log' and 'exp_and_others'
    # sets, paying ~1.28us per switch. Every func we use (Ln, Exp, Square,
    # Copy) lives together in 'natural_log_exp_and_others'; empty the other
    # sets (keeping dict order, so canonical act_func_set_ids are unchanged
    # for walrus) to steer the chooser there once.
    if not getattr(bacc.get_activation_tables, "_attn_patched", False):
        _orig_gat = bacc.get_activation_tables

        def _gat(arch):
            t = _orig_gat(arch)
            keep = "natural_log_exp_and_others"
            if keep not in t:
                return t
            return {k: (v if k == keep else frozenset())
                    for k, v in t.items()}

        _gat._attn_patched = True
        bacc.get_activation_tables = _gat
        hw_specs_gat = getattr(hw_specs, "get_activation_tables", None)
        if hw_specs_gat is not None and not getattr(
                hw_specs_gat, "_attn_patched", False):
            hw_specs.get_activation_tables = _gat

    f32 = mybir.dt.float32
    f32r = mybir.dt.float32r
    bf16 = mybir.dt.bfloat16
    f8 = mybir.dt.float8e4
    DR = mybir.MatmulPerfMode.DoubleRow
    nc = bacc.Bacc("TRN2", target_bir_lowering=False)

    # x layout [kg, base/resid, ko-within-group, p, t] so (base/resid, ko)
    # merge into one 3D-balanceable access pattern per chunk DMA.
    xcat = nc.dram_tensor("xcat", [4, 2, 4, 128, T], f8,
                          kind="ExternalInput")
    # weights ship partition-major so each DMA is one contiguous run per
    # partition (512B+ descriptors: full DMA bus bandwidth in the model)
    qwcat = nc.dram_tensor("qwcat", [128, 2, NKT, HPC * HD], f8,
                           kind="ExternalInput")
    kwcat = nc.dram_tensor("kwcat", [128, 2, NKT, HD], f8,
                           kind="ExternalInput")
    vwcat = nc.dram_tensor("vwcat", [128, 2, NKT, HD], f8,
                           kind="ExternalInput")
    owcat = nc.dram_tensor("owcat", [128, 2, HPC, D], f8,
                           kind="ExternalInput")
    csd = nc.dram_tensor("csd", [128, T], bf16, kind="ExternalInput")
    csd2 = nc.dram_tensor("csd2", [128, T], bf16, kind="ExternalInput")
    normod = nc.dram_tensor("normod", [128, 5, 128], f32r, kind="ExternalInput")
    normbd = nc.dram_tensor("normbd", [128, 5], f32, kind="ExternalInput")
    # [g, p, tb, d]; host reassembles rows g*512 + tb*128 + p.
    outd = nc.dram_tensor("o", [4, 128, 4, D], bf16, kind="ExternalOutput")

    xr = xcat.rearrange("kg two ko p t -> p kg two ko t")

    sq_ = mybir.ActivationFunctionType.Square
    ln_ = mybir.ActivationFunctionType.Ln
    exp_ = mybir.ActivationFunctionType.Exp
    copy_ = mybir.ActivationFunctionType.Copy

    with tile.TileContext(nc) as tc:
        with (
            tc.tile_pool(name="wpool", bufs=1) as wpool,
            tc.tile_pool(name="xpool", bufs=2) as xpool,
            tc.tile_pool(name="big", bufs=1) as big,
            tc.tile_pool(name="vtp", bufs=2) as vtp,
            tc.tile_pool(name="ntmp", bufs=2) as ntmp,
            tc.tile_pool(name="ntm2", bufs=1) as ntm2,
            tc.tile_pool(name="qnp", bufs=6) as qnp,
            tc.tile_pool(name="pjp", bufs=6) as pjp,
            tc.tile_pool(name="padp", bufs=3) as padp,
            tc.tile_pool(name="padd", bufs=3) as paddp,
            tc.tile_pool(name="ttp", bufs=2) as ttp,
            tc.tile_pool(name="y8p", bufs=2) as y8p,
            tc.tile_pool(name="dy8p", bufs=2) as dy8p,
            tc.tile_pool(name="atmp", bufs=2) as atmp,
            tc.tile_pool(name="obp", bufs=1) as obp,
            tc.tile_pool(name="pst2", bufs=2, space="PSUM") as pst2,
            tc.tile_pool(name="py", bufs=2, space="PSUM") as py,
            tc.tile_pool(name="psm", bufs=1, space="PSUM") as psm,
            tc.tile_pool(name="pox", bufs=1, space="PSUM") as pox,
        ):
            # ---- resident weights / tables (DMAs emitted lazily below) ----
            qw_s = wpool.tile([128, 2, NKT, HPC * HD], f8)
            kw_s = wpool.tile([128, 2, NKT, HD], f8)
            vw_s = wpool.tile([128, 2, NKT, HD], f8)
            ow_s = wpool.tile([128, 2, HPC, D], f8)
            cs_s = wpool.tile([128, T], bf16)   # rows 0:64 cos, 64:128 sin
            cs2_s = wpool.tile([128, T], bf16)  # rows 0:64 sin, 64:128 cos
            normo_s = wpool.tile([128, 5, 128], f32r)
            normb_s = wpool.tile([128, 5], f32)
            ones_bf = wpool.tile([128, 64], bf16)
            nc.vector.memset(ones_bf[:], 0.25)
            # masked DR ones: rhs slot 0 (head 0's pad) -> rows 0:64, slot 1
            # (head 1's pad) -> rows 64:128, one DoubleRow matmul for both.
            ones8 = wpool.tile([128, 2, 128], f8)
            nc.vector.memset(ones8[:], 0.0)
            nc.vector.memset(ones8[:, 0, 0:64], 0.25)
            nc.vector.memset(ones8[:, 1, 64:128], 0.25)
            ebias = wpool.tile([128, 1], f32)
            nc.vector.memset(ebias[:], EXP_BIAS)
            zbias = wpool.tile([128, 1], f32)
            nc.vector.memset(zbias[:], 0.0)

            qT = big.tile([128, HPC, T], bf16)
            kT = big.tile([128, T], bf16)
            vtok = big.tile([128, T], bf16)

            def wdma_c0():
                # after chunk0's x: norm constants (qn frees the proj PSUM,
                # so they go early), qw residual, k/v base, k/v residual,
                # rope tables (rope lags freely; qT not needed until attn)
                nc.sync.dma_start(normo_s[:], normod[:])
                nc.sync.dma_start(normb_s[:], normbd[:])
                nc.sync.dma_start(qw_s[:, 1, 0:8], qwcat[:, 1, 0:8])
                nc.sync.dma_start(qw_s[:, 1, 8:16], qwcat[:, 1, 8:16])
                nc.sync.dma_start(kw_s[:, 0], kwcat[:, 0])
                nc.sync.dma_start(vw_s[:, 0], vwcat[:, 0])

                nc.sync.dma_start(kw_s[:, 1], kwcat[:, 1])
                nc.sync.dma_start(vw_s[:, 1], vwcat[:, 1])
                nc.sync.dma_start(cs_s[:], csd[:])
                nc.sync.dma_start(cs2_s[:], csd2[:])

            def wdma_ow():
                nc.sync.dma_start(ow_s[:, 0], owcat[:, 0])
                nc.sync.dma_start(ow_s[:, 1], owcat[:, 1])

            def norm_math(qsb, ni, dst, pos0):
                """qsb: PSUM f32 [128 feat, 512 tok] at 32x scale; ni: 0..3 q
                heads, 4 k. dst: [128, 512] bf16 slice of qT/kT. rmsnorm
                (qg & attn scale folded, 32x absorbed) + rope, hd-major.
                Reads the projection PSUM directly (no SBUF staging copy)."""
                sq = ntmp.tile([128, CHUNK], f32r, tag="sq")
                nc.scalar.activation(out=sq[:], in_=qsb[:], func=sq_)
                nb = pox.tile([128, CHUNK], f32, tag="pox",
                              name=f"nb_{ni}_{pos0}")
                nc.tensor.matmul(nb[:], normo_s[:, ni, :], sq[:],
                                 start=True, stop=True)
                # rfac = rsqrt(nb + eps) = exp(-0.5*ln(.)): keeps every ACT
                # func in the natural_log_exp table set (no table reloads)
                # and needs no DVE reciprocal.
                rs = ntmp.tile([128, CHUNK], f32, tag="rs")
                nc.scalar.activation(out=rs[:], in_=nb[:], func=ln_,
                                     bias=normb_s[:, ni:ni + 1], scale=1.0)
                rfac = ntmp.tile([128, CHUNK], f32, tag="rfac")
                nc.scalar.activation(out=rfac[:], in_=rs[:], func=exp_,
                                     bias=zbias[:], scale=-0.5)
                qn = qnp.tile([128, CHUNK], bf16, tag="qn")
                nc.vector.tensor_mul(qn[:], qsb[:], rfac[:])
                cs = cs_s[0:64, pos0:pos0 + CHUNK]       # cos @ base 0
                sn = cs_s[64:128, pos0:pos0 + CHUNK]     # sin @ base 64
                sn0 = cs2_s[0:64, pos0:pos0 + CHUNK]     # sin @ base 0
                cs64 = cs2_s[64:128, pos0:pos0 + CHUNK]  # cos @ base 64
                t1 = ntm2.tile([64, CHUNK], bf16, tag="ta")
                t2 = ntm2.tile([64, CHUNK], bf16, tag="tb")
                nc.gpsimd.tensor_mul(t1[:], qn[0:64, :], cs)
                nc.vector.tensor_mul(t2[:], qn[64:128, :], sn)
                nc.vector.tensor_add(dst[0:64, :], t1[:], t2[:])
                t3 = ntm2.tile([64, CHUNK], bf16, tag="tc")
                t4 = ntm2.tile([64, CHUNK], bf16, tag="tb")
                nc.vector.tensor_mul(t3[:], qn[0:64, :], sn0)
                nc.vector.tensor_mul(t4[:], qn[64:128, :], cs64)
                nc.vector.tensor_sub(dst[64:128, :], t4[:], t3[:])

            def emit_xdmas(ci, first=False, extra=None):
                t0 = ci * CHUNK
                xt = xpool.tile([128, 4, 2, 4, CHUNK], f8, tag="xt",
                                name=f"xt_{ci}")
                if first:
                    # split by kg so the first matmuls start early, with the
                    # weight DMAs spliced between
                    qwb = [lambda: nc.sync.dma_start(qw_s[:, 0, 0:2],
                                                     qwcat[:, 0, 0:2]),
                           lambda: nc.sync.dma_start(qw_s[:, 0, 2:8],
                                                     qwcat[:, 0, 2:8]),
                           lambda: nc.sync.dma_start(qw_s[:, 0, 8:16],
                                                     qwcat[:, 0, 8:16])]
                    for kg in range(4):
                        nc.sync.dma_start(
                            xt[:, kg], xr[:, kg, :, :, t0:t0 + CHUNK])
                        if kg < 3:
                            qwb[kg]()
                else:
                    nc.sync.dma_start(xt[:], xr[:, :, :, :, t0:t0 + CHUNK])
                if extra is not None:
                    extra()
                return xt

            def proj_pass(xt, w_s, blks, pos0, tagn):
                """One 2-output-block pass of the 3-term fp8 projection.
                blks: two col0 stationary selectors into w_s's last dim.
                Returns the [128,2,CHUNK] psum tile."""
                pq = pst2.tile([128, 2, CHUNK], f32, tag="pst2",
                               name=f"pq_{tagn}_{pos0}")
                for term in range(3):
                    xt_i = 1 if term == 1 else 0
                    w_i = 1 if term == 2 else 0
                    for pr in range(8):
                        rhs = xt[:, pr // 2, xt_i,
                                 2 * (pr % 2):2 * (pr % 2) + 2, :]
                        st = (term == 0 and pr == 0)
                        sp = (term == 2 and pr == 7)
                        for i, c0 in enumerate(blks):
                            lhsT = w_s[:, w_i, 2 * pr:2 * pr + 2,
                                       c0:c0 + 128]
                            nc.tensor.matmul(pq[:, i, :], lhsT, rhs,
                                             start=st, stop=sp,
                                             perf_mode=DR,
                                             skip_group_check=True)
                return pq

            def proj_chunk(ci, first=False, extra=None):
                pos0 = ci * CHUNK
                xt = emit_xdmas(ci, first=first, extra=extra)
                pq = proj_pass(xt, qw_s, (0, 128), pos0, "q01")
                for i in range(2):
                    norm_math(pq[:, i, :], i, qT[:, i, pos0:pos0 + CHUNK],
                              pos0)
                pq = proj_pass(xt, qw_s, (256, 384), pos0, "q23")
                for i in range(2):
                    norm_math(pq[:, i, :], i + 2,
                              qT[:, i + 2, pos0:pos0 + CHUNK], pos0)
                pkv = pst2.tile([128, 2, CHUNK], f32, tag="pst2",
                                name=f"pkv_{ci}")
                for term in range(3):
                    xt_i = 1 if term == 1 else 0
                    w_i = 1 if term == 2 else 0
                    for pr in range(8):
                        rhs = xt[:, pr // 2, xt_i,
                                 2 * (pr % 2):2 * (pr % 2) + 2, :]
                        st = (term == 0 and pr == 0)
                        sp = (term == 2 and pr == 7)
                        nc.tensor.matmul(pkv[:, 0, :],
                                         kw_s[:, w_i, 2 * pr:2 * pr + 2, :],
                                         rhs, start=st, stop=sp,
                                         perf_mode=DR, skip_group_check=True)
                        nc.tensor.matmul(pkv[:, 1, :],
                                         vw_s[:, w_i, 2 * pr:2 * pr + 2, :],
                                         rhs, start=st, stop=sp,
                                         perf_mode=DR, skip_group_check=True)
                vtmp = vtp.tile([128, CHUNK], bf16, tag="vtmp",
                                name=f"vtmp_{ci}")
                nc.scalar.activation(out=vtmp[:], in_=pkv[:, 1, :],
                                     func=copy_, scale=1.0 / WSCALE)
                norm_math(pkv[:, 0, :], 4, kT[:, pos0:pos0 + CHUNK], pos0)
                for tb in range(4):
                    dst0 = pos0 + tb * 128
                    nc.sync.dma_start_transpose(
                        vtok[:, dst0:dst0 + 128],
                        vtmp[:, tb * 128:(tb + 1) * 128])

            y8s = {}

            def attn_pass(g, hp, fillers=()):
                """Scores/exp/pads/pv/sums for heads (2hp, 2hp+1) of query
                group g; finalize writes y8/dy8. fillers: emission closures
                (previous group's oproj units) spread across the pass."""
                kg = 4 * (g + 1)
                q0 = g * CHUNK
                heads = (2 * hp, 2 * hp + 1)
                if hp == 0:
                    y8s[g] = (y8p.tile([128, HPC, CHUNK], f8, tag="y8",
                                       name=f"y8_{g}"),
                              dy8p.tile([128, HPC, CHUNK], f8, tag="dy8",
                                        name=f"dy8_{g}"))
                y8t, dy8t = y8s[g]
                ys = {}
                for i, h in enumerate(heads):
                    ys[i] = py.tile([128, CHUNK], f32, tag="py",
                                    name=f"y_{g}_{hp}_{i}")
                smt = psm.tile([128, CHUNK], f32, tag="psm",
                               name=f"sm_{g}_{hp}")
                sms = [smt[0:64, :], smt[64:128, :]]

                n_far = 2 * g            # far j-pairs per head
                stepn = [0]
                n_steps = (n_far + 2) * 2 + 2
                fi = [0]

                def fill():
                    stepn[0] += 1
                    # lag the first fillers so the previous group's y8/dy8
                    # finalize chain has drained before oproj units need it
                    eff = max(0, stepn[0] - 2)
                    want = min(len(fillers),
                               (eff * len(fillers)) // max(1, n_steps - 2)
                               + (1 if eff > 0 else 0))
                    while fi[0] < want:
                        fillers[fi[0]]()
                        fi[0] += 1

                pend_pv = []
                pend_sums = []
                sums_started = [False, False]

                def emit_pv(unit):
                    jj0, pjt, i, c0s = unit
                    for jj in range(2):
                        j = jj0 + jj
                        nc.tensor.matmul(
                            ys[i][:, c0s[jj]:],
                            vtok[:, j * 128:(j + 1) * 128],
                            pjt[:, jj, c0s[jj]:],
                            start=(j == 0), stop=(j == kg - 1),
                            skip_group_check=True)

                def emit_sums(unit):
                    kind, i, op, c0 = unit
                    if kind == "dr":
                        # writes both heads' 64-row ranges at once
                        st = not (sums_started[0] or sums_started[1])
                        sums_started[0] = sums_started[1] = True
                        nc.tensor.matmul(smt[:, c0:], ones8[:],
                                         op[:, :, c0:], start=st, stop=False,
                                         perf_mode=DR, skip_group_check=True)
                    else:
                        st = not sums_started[i]
                        sums_started[i] = True
                        nc.tensor.matmul(sms[i][:, c0:], ones_bf[:],
                                         op[:, c0:], start=st,
                                         stop=(kind == "last"),
                                         skip_group_check=True)

                def drain(pv_keep=1, sums_keep=1):
                    while len(pend_pv) > pv_keep:
                        emit_pv(pend_pv.pop(0))
                    while len(pend_sums) > sums_keep:
                        emit_sums(pend_sums.pop(0))

                padq = {}
                # far pairs: j = 2p, 2p+1; fully causal, full query width
                for p in range(n_far):
                    for i, h in enumerate(heads):
                        st2 = pst2.tile([128, 2, CHUNK], f32, tag="pst2",
                                        name=f"st_{g}_{hp}_{p}_{i}")
                        for jj in range(2):
                            j = 2 * p + jj
                            nc.tensor.matmul(
                                st2[:, jj, :],
                                kT[:, j * 128:(j + 1) * 128],
                                qT[:, h, q0:q0 + CHUNK],
                                start=True, stop=True,
                                skip_group_check=True)
                        drain()
                        pjt = pjp.tile([128, 2, CHUNK], bf16, tag="pj",
                                       name=f"pj_{g}_{hp}_{p}_{i}")
                        nc.scalar.activation(out=pjt[:], in_=st2[:],
                                             func=exp_, bias=ebias[:],
                                             scale=1.0)
                        if i == 0:
                            padq[p] = padp.tile([128, 2, CHUNK], f8,
                                                tag="padq",
                                                name=f"pq_{g}_{hp}_{p}")
                        nc.vector.tensor_add(padq[p][:, i, :],
                                             pjt[:, 0, :], pjt[:, 1, :])
                        if i == 1:
                            pend_sums.append(("dr", 0, padq[p], 0))
                        pend_pv.append((2 * p, pjt, i, (0, 0)))
                        fill()
                # diagonal pairs: j = 4g+2dp, 4g+2dp+1
                for dp in range(2):
                    for i, h in enumerate(heads):
                        j0 = 4 * g + 2 * dp
                        c00, c01 = 256 * dp, 256 * dp + 128
                        st2 = pst2.tile([128, 2, CHUNK], f32, tag="pst2",
                                        name=f"std_{g}_{hp}_{dp}_{i}")
                        nc.tensor.matmul(st2[:, 0, c00:],
                                         kT[:, j0 * 128:(j0 + 1) * 128],
                                         qT[:, h, q0 + c00:q0 + CHUNK],
                                         start=True, stop=True,
                                         skip_group_check=True)
                        nc.tensor.matmul(st2[:, 1, c01:],
                                         kT[:, (j0 + 1) * 128:(j0 + 2) * 128],
                                         qT[:, h, q0 + c01:q0 + CHUNK],
                                         start=True, stop=True,
                                         skip_group_check=True)
                        drain()
                        pjt = pjp.tile([128, 2, CHUNK], bf16, tag="pj",
                                       name=f"pjd_{g}_{hp}_{dp}_{i}")
                        nc.scalar.activation(out=pjt[:, 0, c00:],
                                             in_=st2[:, 0, c00:],
                                             func=exp_, bias=ebias[:],
                                             scale=1.0)
                        nc.scalar.activation(out=pjt[:, 1, c01:],
                                             in_=st2[:, 1, c01:],
                                             func=exp_, bias=ebias[:],
                                             scale=1.0)
                        nc.vector.memset(pjt[:, 1, c00:c01], 0.0)
                        for jj, cc in ((0, c00), (1, c01)):
                            nc.gpsimd.affine_select(
                                out=pjt[:, jj, cc:cc + 128],
                                in_=pjt[:, jj, cc:cc + 128],
                                pattern=[[1, 128]],
                                compare_op=mybir.AluOpType.is_ge,
                                fill=0.0,
                                base=0,
                                channel_multiplier=-1)
                        pdd = paddp.tile([128, CHUNK], bf16, tag="padd",
                                         name=f"pd_{g}_{hp}_{dp}_{i}")
                        nc.vector.tensor_add(pdd[:, c00:], pjt[:, 0, c00:],
                                             pjt[:, 1, c00:])
                        pend_sums.append(("last" if dp == 1 else "bf",
                                          i, pdd, c00))
                        pend_pv.append((j0, pjt, i, (c00, c01)))
                        fill()
                drain(0, 0)
                # finalize: r64 = 4/denominator; y8/dy8 split for oproj
                for i, h in enumerate(heads):
                    r64 = atmp.tile([64, CHUNK], f32, tag="r64",
                                    name=f"r_{g}_{hp}_{i}")
                    nc.vector.reciprocal(r64[:], sms[i])
                    t = ttp.tile([128, CHUNK], f32, tag="t",
                                 name=f"t_{g}_{hp}_{i}")
                    for half in range(2):
                        p0 = 64 * half
                        tsl = t[p0:p0 + 64, :]
                        nc.vector.tensor_mul(tsl, ys[i][p0:p0 + 64, :],
                                             r64[:])
                        y8sl = y8t[p0:p0 + 64, h, :]
                        if g == 3:
                            # tail: DVE is idle and faster than the Q7 chain
                            nc.vector.tensor_copy(out=y8sl, in_=tsl)
                            nc.vector.tensor_sub(dy8t[p0:p0 + 64, h, :],
                                                 tsl, y8sl)
                        else:
                            nc.gpsimd.tensor_copy(out=y8sl, in_=tsl)
                            nc.gpsimd.tensor_sub(dy8t[p0:p0 + 64, h, :],
                                                 tsl, y8sl)
                    fill()
                while fi[0] < len(fillers):
                    fillers[fi[0]]()
                    fi[0] += 1

            def oproj_units(g, alt=False, split_store=False):
                """16 oproj unit closures + final store; 3-term fp8
                DoubleRow over head pairs. alt: alternate pox/py pools
                (tail group, attention pools free)."""
                obuf = obp.tile([128, 4, D], bf16, tag="obuf",
                                name=f"ob_{g}")
                y8t, dy8t = y8s[g]
                units = []

                def unit(tb, oc, n):
                    def go():
                        pool = py if (alt and n % 2 == 1) else pox
                        po = pool.tile([128, CHUNK], f32,
                                       tag="py" if (alt and n % 2 == 1)
                                       else "pox",
                                       name=f"op_{g}_{tb}_{oc}")
                        first, last = True, None
                        mms = []
                        for hp in range(2):
                            h0 = 2 * hp
                            for term in range(3):
                                ysl = dy8t if term == 1 else y8t
                                w_i = 1 if term == 2 else 0
                                mms.append((
                                    ysl[:, h0:h0 + 2,
                                        tb * 128:(tb + 1) * 128],
                                    ow_s[:, w_i, h0:h0 + 2,
                                         oc * CHUNK:(oc + 1) * CHUNK]))
                        for mi, (lhsT, rhs) in enumerate(mms):
                            nc.tensor.matmul(po[:], lhsT, rhs,
                                             start=(mi == 0),
                                             stop=(mi == len(mms) - 1),
                                             perf_mode=DR,
                                             skip_group_check=True)
                        dst = obuf[:, tb, oc * CHUNK:(oc + 1) * CHUNK]
                        if n % 3 != 2:
                            nc.scalar.activation(out=dst, in_=po[:],
                                                 func=copy_,
                                                 scale=1.0 / 128.0)
                        else:
                            nc.vector.tensor_scalar_mul(dst, po[:],
                                                        1.0 / 128.0)
                        if split_store and oc == 3:
                            nc.sync.dma_start(outd[g, :, tb, :],
                                              obuf[:, tb, :])
                    return go

                n = 0
                for tb in range(4):
                    for oc in range(4):
                        units.append(unit(tb, oc, n))
                        n += 1
                if not split_store:
                    def store():
                        nc.sync.dma_start(outd[g], obuf[:])
                    units.append(store)
                return units

            # ---- schedule ----
            proj_chunk(0, first=True, extra=wdma_c0)
            proj_chunk(1)
            proj_chunk(2, extra=wdma_ow)
            attn_pass(0, 0)
            proj_chunk(3)
            attn_pass(0, 1)
            attn_pass(1, 0, fillers=oproj_units(0))
            attn_pass(1, 1)
            attn_pass(2, 0, fillers=oproj_units(1))
            attn_pass(2, 1)
            attn_pass(3, 0, fillers=oproj_units(2))
            attn_pass(3, 1)
            for u in oproj_units(3, alt=True, split_store=True):
                u()

    nc.compile()
    return nc


_CACHED = {}
LAST_EXEC_NS = None


def _run(nc, in_maps, **kwargs):
    from concourse.bass_utils import run_bass_kernel_spmd
    return run_bass_kernel_spmd(nc, in_maps, core_ids=list(range(NCORES)),
                                **kwargs)


def _make_in_maps(x, qw, kw, vw, ow, qg):
    import ml_dtypes
    bf = ml_dtypes.bfloat16
    f8 = ml_dtypes.float8_e4m3

    def split8(a, scale):
        a32 = np.asarray(a, np.float32) * np.float32(scale)
        a8 = a32.astype(f8)
        da = (a32 - a8.astype(np.float32)).astype(f8)
        return np.stack([a8, da])  # [2, ...]

    cosT, sinT = _rope_tables()
    cossin = np.concatenate([cosT, sinT], axis=0).astype(bf)   # [128, T]
    sincos = np.concatenate([sinT, cosT], axis=0).astype(bf)   # [128, T]

    def xlayout(xb):
        s = split8(xb.reshape(T, D).T, 1.0)        # [2, D, T]
        return np.ascontiguousarray(
            s.reshape(2, 4, 4, 128, T).transpose(1, 0, 2, 3, 4))

    xcats = [xlayout(x[b]) for b in range(B)]

    in_maps = []
    for c in range(NCORES):
        b, kvg = c // 4, c % 4
        h0 = HPC * kvg
        def pmajor(a, m):
            # [2, D_or_512, m] -> [128, 2, ko, m] partition-major
            return np.ascontiguousarray(
                a.reshape(2, -1, 128, m).transpose(2, 0, 1, 3))

        qwcat = pmajor(split8(qw[h0 * HD:(h0 + HPC) * HD, :].T, WSCALE), 512)
        kwcat = pmajor(split8(kw[kvg * HD:(kvg + 1) * HD, :].T, WSCALE), HD)
        vwcat = pmajor(split8(vw[kvg * HD:(kvg + 1) * HD, :].T, WSCALE), HD)
        owcat = pmajor(split8(ow[:, h0 * HD:(h0 + HPC) * HD].T, WSCALE), D)
        # s_i folds qg gain and 1/sqrt(HD); 32x psum scale cancels in the
        # ratio, only the EPS bias needs the 32^2 factor.
        s = np.array([qg[h0] / np.sqrt(HD), qg[h0 + 1] / np.sqrt(HD),
                      qg[h0 + 2] / np.sqrt(HD), qg[h0 + 3] / np.sqrt(HD),
                      1.0], np.float32)
        normo = np.broadcast_to(
            (1.0 / (HD * s * s))[None, :, None], (128, 5, 128)
        ).astype(np.float32).copy()
        normb = np.broadcast_to(
            (EPS * WSCALE * WSCALE / (s * s))[None, :],
            (128, 5)).astype(np.float32).copy()
        in_maps.append({
            "xcat": xcats[b],
            "qwcat": np.ascontiguousarray(qwcat),
            "kwcat": np.ascontiguousarray(kwcat),
            "vwcat": np.ascontiguousarray(vwcat),
            "owcat": np.ascontiguousarray(owcat),
            "csd": cossin, "csd2": sincos,
            "normod": normo, "normbd": normb,
        })
    return in_maps


def kernel(x, qw, kw, vw, ow, qg):
    global LAST_EXEC_NS
    x = np.ascontiguousarray(x, dtype=np.float32)
    qw = np.asarray(qw, dtype=np.float32)
    kw = np.asarray(kw, dtype=np.float32)
    vw = np.asarray(vw, dtype=np.float32)
    ow = np.asarray(ow, dtype=np.float32)
    qg = np.asarray(qg, dtype=np.float32)

    if "nc" not in _CACHED:
        _CACHED["nc"] = _build_program()
    nc = _CACHED["nc"]

    in_maps = _make_in_maps(x, qw, kw, vw, ow, qg)
    res = _run(nc, in_maps)
    LAST_EXEC_NS = res.exec_time_ns
    out = np.empty((B, T, D), np.float32)
    for b in range(B):
        acc = np.zeros((4, 128, 4, D), np.float32)
        for kvg in range(4):
            acc += res.results[4 * b + kvg]["o"].astype(np.float32)
        # [g, p, tb, d] -> rows g*512 + tb*128 + p
        out[b] = acc.transpose(0, 2, 1, 3).reshape(T, D)
    return np.ascontiguousarray(out)


# revision 29
# speedup vs baseline: 1.2972x; 1.0027x over previous
"""Bass/Tile kernel for nn_Attn_40424232189956 on 8 trn2 NeuronCores.

GQA attention block: q/k/v proj + rmsnorm + rope + causal attention + out proj.
B=2, T=2048, D=2048, NH=16, NKV=4, HD=128.

Sharding: core c -> (batch b = c//4, kv-group kvg = c%4). Each core owns one
batch's tokens, q heads 4*kvg..4*kvg+3 and kv head kvg; it computes a full
[T, D] partial of the output projection and the host sums the 4 partials per
batch. Unlike the head-only sharding this removes all duplicated k/v
projection work and halves both the x load and the output store traffic.

Performance structure (tuned against the TimelineSim cost model):
- Projections and the output projection run as THREE fp8 (e4m3) DoubleRow
  matmul terms: a8@b8 + da8@b8 + a8@db8, where da/db are the fp8 residuals
  of the bf16-class value. DoubleRow processes two 128-contraction tiles per
  instruction at 0.5 cycles/moving-column in the cost model, so the 3-term
  split costs 0.75x of the bf16 pair while keeping ~bf16 accuracy (the
  dropped da@db term and residual quantization are ~0.1% effects).
  Weights ship 32-scaled (fp8 subnormal cutoff), x unscaled; the rmsnorm
  absorbs the 32x PSUM scale for q/k (EPS bias scaled by 1024), the v copy
  divides by 32, and the oproj staging copy divides by 128 (=4*32, the 4
  from folding 0.25 into the softmax-denominator ones matmul).
- Attention (scores, exp, pv) stays bf16: fp8 there fails the 2e-2 gate.
  Far (fully-causal) key-block pairs share one [128,2,512] PSUM stile and a
  single [128,1024] exp; diagonal blocks keep windowed exps + gpsimd
  affine_select masking. exp carries a -2 bias (cancelled by the softmax
  normalization) so pj and the fp8 pads stay inside e4m3 range.
- Softmax denominators: far pad pairs are fp8 and summed with a DoubleRow
  ones matmul (4x cheaper); diagonal pads stay bf16 ones-matmuls so the
  short early rows keep bf16-accurate denominators.
- Normalized attention output is split y8+dy8 (fp8 + fp8 residual) on
  DVE/gpsimd for the 3-term oproj; r64 multiplies by 4/denominator.
- oproj units (16 per group) are interleaved into the NEXT group's first
  attention pass so the PE never waits on the normalize chain; the last
  group alternates two PSUM pools and stores per-128-token-block.
- Engine balance: ACT = qsb/v copies, Square, Sqrt, exp, half the staging
  copies (sqrt and exp eras separated to minimize act-table loads); DVE =
  rope mixes (bf16 2x), qn, reciprocals, pad adds, t-muls, other staging
  copies; gpsimd = one rope mix, affine_select, y8 quantize + dy residual.
"""

import numpy as np

B, T, D = 2, 2048, 2048
NH, NKV = 16, 4
HD = 128
NCORES = 8
HPC = 4               # q heads per core
NKT = D // 128        # 16 contraction tiles for projections
CHUNK = 512
EPS = float(np.finfo(np.float32).eps)
EXP_BIAS = -2.0
WSCALE = 32.0


def _rope_tables():
    # Matches reference.rotary_tables for T=2048 > tsl=1024 (NTK branch).
    hd = np.float32(HD)
    ar = (np.arange(0, HD, 2, dtype=np.float32) / hd).astype(np.float32)
    expo = np.power(np.float32(HD / (HD - 2.0)), ar, dtype=np.float32)
    inv = (np.float32(1.0)
           / (np.float32(10000.0)
              * np.power(np.float32(T / 1024.0), expo, dtype=np.float32)))
    f = np.outer(np.arange(T, dtype=np.float32), inv.astype(np.float32))
    return (np.cos(f).astype(np.float32).T.copy(),
            np.sin(f).astype(np.float32).T.copy())  # [64, T] hd-major


def _build_program():
    import concourse.bass as bass
    import concourse.mybir as mybir
    import concourse.tile as tile
    from concourse import bacc
    import concourse.hw_specs as hw_specs

    # The act-table-load inserter picks the FIRST table set containing each
    # activation func, so a kernel mixing Ln (rmsnorm rsqrt) and Exp
    # (softmax) flip-flops between the 'natural_log' and 'exp_and_others'
    # sets, paying ~1.28us per switch. Every func we use (Ln, Exp, Square,
    # Copy) lives together in 'natural_log_exp_and_others'; empty the other
    # sets (keeping dict order, so canonical act_func_set_ids are unchanged
    # for walrus) to steer the chooser there once.
    if not getattr(bacc.get_activation_tables, "_attn_patched", False):
        _orig_gat = bacc.get_activation_tables

        def _gat(arch):
            t = _orig_gat(arch)
            keep = "natural_log_exp_and_others"
            if keep not in t:
                return t
            return {k: (v if k == keep else frozenset())
                    for k, v in t.items()}

        _gat._attn_patched = True
        bacc.get_activation_tables = _gat
        hw_specs_gat = getattr(hw_specs, "get_activation_tables", None)
        if hw_specs_gat is not None and not getattr(
                hw_specs_gat, "_attn_patched", False):
            hw_specs.get_activation_tables = _gat

    f32 = mybir.dt.float32
    f32r = mybir.dt.float32r
    bf16 = mybir.dt.bfloat16
    f8 = mybir.dt.float8e4
    DR = mybir.MatmulPerfMode.DoubleRow
    nc = bacc.Bacc("TRN2", target_bir_lowering=False)

    # x layout [kg, base/resid, ko-within-group, p, t] so (base/resid, ko)
    # merge into one 3D-balanceable access pattern per chunk DMA.
    xcat = nc.dram_tensor("xcat", [4, 2, 4, 128, T], f8,
                          kind="ExternalInput")
    # weights ship partition-major so each DMA is one contiguous run per
    # partition (512B+ descriptors: full DMA bus bandwidth in the model)
    qwcat = nc.dram_tensor("qwcat", [128, 2, NKT, HPC * HD], f8,
                           kind="ExternalInput")
    kwcat = nc.dram_tensor("kwcat", [128, 2, NKT, HD], f8,
                           kind="ExternalInput")
    vwcat = nc.dram_tensor("vwcat", [128, 2, NKT, HD], f8,
                           kind="ExternalInput")
    owcat = nc.dram_tensor("owcat", [128, 2, HPC, D], f8,
                           kind="ExternalInput")
    csd = nc.dram_tensor("csd", [128, T], bf16, kind="ExternalInput")
    csd2 = nc.dram_tensor("csd2", [128, T], bf16, kind="ExternalInput")
    normod = nc.dram_tensor("normod", [128, 5, 128], f32r, kind="ExternalInput")
    normbd = nc.dram_tensor("normbd", [128, 5], f32, kind="ExternalInput")
    # [g, p, tb, d]; host reassembles rows g*512 + tb*128 + p.
    outd = nc.dram_tensor("o", [4, 128, 4, D], bf16, kind="ExternalOutput")

    xr = xcat.rearrange("kg two ko p t -> p kg two ko t")

    sq_ = mybir.ActivationFunctionType.Square
    ln_ = mybir.ActivationFunctionType.Ln
    exp_ = mybir.ActivationFunctionType.Exp
    copy_ = mybir.ActivationFunctionType.Copy

    with tile.TileContext(nc) as tc:
        with (
            tc.tile_pool(name="wpool", bufs=1) as wpool,
            tc.tile_pool(name="xpool", bufs=2) as xpool,
            tc.tile_pool(name="big", bufs=1) as big,
            tc.tile_pool(name="vtp", bufs=2) as vtp,
            tc.tile_pool(name="ntmp", bufs=2) as ntmp,
            tc.tile_pool(name="ntm2", bufs=1) as ntm2,
            tc.tile_pool(name="qnp", bufs=10) as qnp,
            tc.tile_pool(name="pjp", bufs=6) as pjp,
            tc.tile_pool(name="padp", bufs=3) as padp,
            tc.tile_pool(name="padd", bufs=3) as paddp,
            tc.tile_pool(name="ttp", bufs=2) as ttp,
            tc.tile_pool(name="y8p", bufs=2) as y8p,
            tc.tile_pool(name="dy8p", bufs=2) as dy8p,
            tc.tile_pool(name="atmp", bufs=2) as atmp,
            tc.tile_pool(name="obp", bufs=1) as obp,
            tc.tile_pool(name="pst2", bufs=2, space="PSUM") as pst2,
            tc.tile_pool(name="py", bufs=2, space="PSUM") as py,
            tc.tile_pool(name="psm", bufs=1, space="PSUM") as psm,
            tc.tile_pool(name="pox", bufs=1, space="PSUM") as pox,
        ):
            # ---- resident weights / tables (DMAs emitted lazily below) ----
            qw_s = wpool.tile([128, 2, NKT, HPC * HD], f8)
            kw_s = wpool.tile([128, 2, NKT, HD], f8)
            vw_s = wpool.tile([128, 2, NKT, HD], f8)
            ow_s = wpool.tile([128, 2, HPC, D], f8)
            cs_s = wpool.tile([128, T], bf16)   # rows 0:64 cos, 64:128 sin
            cs2_s = wpool.tile([128, T], bf16)  # rows 0:64 sin, 64:128 cos
            normo_s = wpool.tile([128, 5, 128], f32r)
            normb_s = wpool.tile([128, 5], f32)
            ones_bf = wpool.tile([128, 64], bf16)
            nc.vector.memset(ones_bf[:], 0.25)
            # masked DR ones: rhs slot 0 (head 0's pad) -> rows 0:64, slot 1
            # (head 1's pad) -> rows 64:128, one DoubleRow matmul for both.
            ones8 = wpool.tile([128, 2, 128], f8)
            nc.vector.memset(ones8[:], 0.0)
            nc.vector.memset(ones8[:, 0, 0:64], 0.25)
            nc.vector.memset(ones8[:, 1, 64:128], 0.25)
            ebias = wpool.tile([128, 1], f32)
            nc.vector.memset(ebias[:], EXP_BIAS)
            zbias = wpool.tile([128, 1], f32)
            nc.vector.memset(zbias[:], 0.0)

            qT = big.tile([128, HPC, T], bf16)
            kT = big.tile([128, T], bf16)
            vtok = big.tile([128, T], bf16)

            def wdma_ow():
                nc.sync.dma_start(ow_s[:, 0], owcat[:, 0])
                nc.sync.dma_start(ow_s[:, 1], owcat[:, 1])

            def norm_a(qsb, ni, pos0, tag):
                """Eager rmsnorm front half reading the projection PSUM
                directly: sq -> nb -> ln -> exp(-0.5) -> qn. Returns the qn
                tile; qn frees the PSUM slice."""
                sq = ntmp.tile([128, CHUNK], f32r, tag="sq")
                nc.scalar.activation(out=sq[:], in_=qsb[:], func=sq_)
                nb = pox.tile([128, CHUNK], f32, tag="pox",
                              name=f"nb_{ni}_{pos0}")
                nc.tensor.matmul(nb[:], normo_s[:, ni, :], sq[:],
                                 start=True, stop=True)
                # rfac = rsqrt(nb + eps) = exp(-0.5*ln(.)): keeps every ACT
                # func in the natural_log_exp table set (no table reloads)
                # and needs no DVE reciprocal.
                rs = ntmp.tile([128, CHUNK], f32, tag="rs")
                nc.scalar.activation(out=rs[:], in_=nb[:], func=ln_,
                                     bias=normb_s[:, ni:ni + 1], scale=1.0)
                rfac = ntmp.tile([128, CHUNK], f32, tag="rfac")
                nc.scalar.activation(out=rfac[:], in_=rs[:], func=exp_,
                                     bias=zbias[:], scale=-0.5)
                qn = qnp.tile([128, CHUNK], bf16, tag="qn", name=f"qn_{tag}")
                nc.vector.tensor_mul(qn[:], qsb[:], rfac[:])
                return qn

            def rope_b(qn, dst, pos0):
                """Deferred rope mix: reads qn (SBUF) and the cos/sin
                tables; writes the bf16 qT/kT slice."""
                cs = cs_s[0:64, pos0:pos0 + CHUNK]       # cos @ base 0
                sn = cs_s[64:128, pos0:pos0 + CHUNK]     # sin @ base 64
                sn0 = cs2_s[0:64, pos0:pos0 + CHUNK]     # sin @ base 0
                cs64 = cs2_s[64:128, pos0:pos0 + CHUNK]  # cos @ base 64
                t1 = ntm2.tile([64, CHUNK], bf16, tag="ta")
                t2 = ntm2.tile([64, CHUNK], bf16, tag="tb")
                nc.gpsimd.tensor_mul(t1[:], qn[0:64, :], cs)
                nc.vector.tensor_mul(t2[:], qn[64:128, :], sn)
                nc.vector.tensor_add(dst[0:64, :], t1[:], t2[:])
                t3 = ntm2.tile([64, CHUNK], bf16, tag="tc")
                t4 = ntm2.tile([64, CHUNK], bf16, tag="tb")
                nc.vector.tensor_mul(t3[:], qn[0:64, :], sn0)
                nc.vector.tensor_mul(t4[:], qn[64:128, :], cs64)
                nc.vector.tensor_sub(dst[64:128, :], t4[:], t3[:])

            def emit_xdmas(ci, first=False):
                t0 = ci * CHUNK
                xt = xpool.tile([128, 4, 2, 4, CHUNK], f8, tag="xt",
                                name=f"xt_{ci}")
                if first:
                    qwb = [lambda: nc.sync.dma_start(qw_s[:, 0, 0:2],
                                                     qwcat[:, 0, 0:2]),
                           lambda: nc.sync.dma_start(qw_s[:, 0, 2:8],
                                                     qwcat[:, 0, 2:8]),
                           lambda: nc.sync.dma_start(qw_s[:, 0, 8:16],
                                                     qwcat[:, 0, 8:16])]
                for kg in range(4):
                    nc.sync.dma_start(
                        xt[:, kg], xr[:, kg, :, :, t0:t0 + CHUNK])
                    if first and kg < 3:
                        qwb[kg]()
                if first:
                    nc.sync.dma_start(normo_s[:], normod[:])
                    nc.sync.dma_start(normb_s[:], normbd[:])
                    nc.sync.dma_start(qw_s[:, 1, 0:8], qwcat[:, 1, 0:8])
                    nc.sync.dma_start(qw_s[:, 1, 8:16], qwcat[:, 1, 8:16])
                return xt

            def proj_pass(xt, w_s, blks, pos0, tagn):
                """One 2-output-block pass of the 3-term fp8 projection.
                blks: two col0 stationary selectors into w_s's last dim.
                Returns the [128,2,CHUNK] psum tile."""
                pq = pst2.tile([128, 2, CHUNK], f32, tag="pst2",
                               name=f"pq_{tagn}_{pos0}")
                for term in range(3):
                    xt_i = 1 if term == 1 else 0
                    w_i = 1 if term == 2 else 0
                    for pr in range(8):
                        rhs = xt[:, pr // 2, xt_i,
                                 2 * (pr % 2):2 * (pr % 2) + 2, :]
                        st = (term == 0 and pr == 0)
                        sp = (term == 2 and pr == 7)
                        for i, c0 in enumerate(blks):
                            lhsT = w_s[:, w_i, 2 * pr:2 * pr + 2,
                                       c0:c0 + 128]
                            nc.tensor.matmul(pq[:, i, :], lhsT, rhs,
                                             start=st, stop=sp,
                                             perf_mode=DR,
                                             skip_group_check=True)
                return pq

            def proj_chunk(ci, xt, nxt=None):
                """Passes + eager norm-A inline; prefetches the next chunk's
                x; rope-B and v transposes at the tail. Chunk 0 also places
                the k/v and table DMAs at their deadline positions."""
                pos0 = ci * CHUNK
                qns = []
                pq = proj_pass(xt, qw_s, (0, 128), pos0, "q01")
                for i in range(2):
                    qns.append(norm_a(pq[:, i, :], i, pos0, f"{ci}_{i}"))
                pq = proj_pass(xt, qw_s, (256, 384), pos0, "q23")
                for i in range(2):
                    qns.append(norm_a(pq[:, i, :], i + 2, pos0,
                                      f"{ci}_{i+2}"))
                if ci == 0:
                    nc.sync.dma_start(kw_s[:, 0], kwcat[:, 0])
                    nc.sync.dma_start(vw_s[:, 0], vwcat[:, 0])
                    nc.sync.dma_start(kw_s[:, 1], kwcat[:, 1])
                    nc.sync.dma_start(vw_s[:, 1], vwcat[:, 1])
                pkv = pst2.tile([128, 2, CHUNK], f32, tag="pst2",
                                name=f"pkv_{ci}")
                for term in range(3):
                    xt_i = 1 if term == 1 else 0
                    w_i = 1 if term == 2 else 0
                    for pr in range(8):
                        rhs = xt[:, pr // 2, xt_i,
                                 2 * (pr % 2):2 * (pr % 2) + 2, :]
                        st = (term == 0 and pr == 0)
                        sp = (term == 2 and pr == 7)
                        nc.tensor.matmul(pkv[:, 0, :],
                                         kw_s[:, w_i, 2 * pr:2 * pr + 2, :],
                                         rhs, start=st, stop=sp,
                                         perf_mode=DR, skip_group_check=True)
                        nc.tensor.matmul(pkv[:, 1, :],
                                         vw_s[:, w_i, 2 * pr:2 * pr + 2, :],
                                         rhs, start=st, stop=sp,
                                         perf_mode=DR, skip_group_check=True)
                vtmp = vtp.tile([128, CHUNK], bf16, tag="vtmp",
                                name=f"vtmp_{ci}")
                nc.scalar.activation(out=vtmp[:], in_=pkv[:, 1, :],
                                     func=copy_, scale=1.0 / WSCALE)
                qns.append(norm_a(pkv[:, 0, :], 4, pos0, f"{ci}_k"))
                xt_next = emit_xdmas(ci + 1) if nxt else None
                if ci == 0:
                    nc.sync.dma_start(cs_s[:], csd[:])
                    nc.sync.dma_start(cs2_s[:], csd2[:])
                if ci == 2:
                    wdma_ow()
                for i in range(4):
                    rope_b(qns[i], qT[:, i, pos0:pos0 + CHUNK], pos0)
                rope_b(qns[4], kT[:, pos0:pos0 + CHUNK], pos0)
                for tb in range(4):
                    dst0 = pos0 + tb * 128
                    nc.sync.dma_start_transpose(
                        vtok[:, dst0:dst0 + 128],
                        vtmp[:, tb * 128:(tb + 1) * 128])
                return xt_next

            y8s = {}

            def attn_pass(g, hp, fillers=()):
                """Scores/exp/pads/pv/sums for heads (2hp, 2hp+1) of query
                group g; finalize writes y8/dy8. fillers: emission closures
                (previous group's oproj units) spread across the pass."""
                kg = 4 * (g + 1)
                q0 = g * CHUNK
                heads = (2 * hp, 2 * hp + 1)
                if hp == 0:
                    y8s[g] = (y8p.tile([128, HPC, CHUNK], f8, tag="y8",
                                       name=f"y8_{g}"),
                              dy8p.tile([128, HPC, CHUNK], f8, tag="dy8",
                                        name=f"dy8_{g}"))
                y8t, dy8t = y8s[g]
                ys = {}
                for i, h in enumerate(heads):
                    ys[i] = py.tile([128, CHUNK], f32, tag="py",
                                    name=f"y_{g}_{hp}_{i}")
                smt = psm.tile([128, CHUNK], f32, tag="psm",
                               name=f"sm_{g}_{hp}")
                sms = [smt[0:64, :], smt[64:128, :]]

                n_far = 2 * g            # far j-pairs per head
                stepn = [0]
                n_steps = (n_far + 2) * 2 + 2
                fi = [0]

                def fill():
                    stepn[0] += 1
                    # lag the first fillers so the previous group's y8/dy8
                    # finalize chain has drained before oproj units need it
                    eff = max(0, stepn[0] - 2)
                    want = min(len(fillers),
                               (eff * len(fillers)) // max(1, n_steps - 2)
                               + (1 if eff > 0 else 0))
                    while fi[0] < want:
                        fillers[fi[0]]()
                        fi[0] += 1

                pend_pv = []
                pend_sums = []
                sums_started = [False, False]

                def emit_pv(unit):
                    jj0, pjt, i, c0s = unit
                    for jj in range(2):
                        j = jj0 + jj
                        nc.tensor.matmul(
                            ys[i][:, c0s[jj]:],
                            vtok[:, j * 128:(j + 1) * 128],
                            pjt[:, jj, c0s[jj]:],
                            start=(j == 0), stop=(j == kg - 1),
                            skip_group_check=True)

                def emit_sums(unit):
                    kind, i, op, c0 = unit
                    if kind == "dr":
                        # writes both heads' 64-row ranges at once
                        st = not (sums_started[0] or sums_started[1])
                        sums_started[0] = sums_started[1] = True
                        nc.tensor.matmul(smt[:, c0:], ones8[:],
                                         op[:, :, c0:], start=st, stop=False,
                                         perf_mode=DR, skip_group_check=True)
                    else:
                        st = not sums_started[i]
                        sums_started[i] = True
                        nc.tensor.matmul(sms[i][:, c0:], ones_bf[:],
                                         op[:, c0:], start=st,
                                         stop=(kind == "last"),
                                         skip_group_check=True)

                def drain(pv_keep=1, sums_keep=1):
                    while len(pend_pv) > pv_keep:
                        emit_pv(pend_pv.pop(0))
                    while len(pend_sums) > sums_keep:
                        emit_sums(pend_sums.pop(0))

                padq = {}
                # far pairs: j = 2p, 2p+1; fully causal, full query width
                for p in range(n_far):
                    for i, h in enumerate(heads):
                        st2 = pst2.tile([128, 2, CHUNK], f32, tag="pst2",
                                        name=f"st_{g}_{hp}_{p}_{i}")
                        for jj in range(2):
                            j = 2 * p + jj
                            nc.tensor.matmul(
                                st2[:, jj, :],
                                kT[:, j * 128:(j + 1) * 128],
                                qT[:, h, q0:q0 + CHUNK],
                                start=True, stop=True,
                                skip_group_check=True)
                        drain()
                        pjt = pjp.tile([128, 2, CHUNK], bf16, tag="pj",
                                       name=f"pj_{g}_{hp}_{p}_{i}")
                        nc.scalar.activation(out=pjt[:], in_=st2[:],
                                             func=exp_, bias=ebias[:],
                                             scale=1.0)
                        if i == 0:
                            padq[p] = padp.tile([128, 2, CHUNK], f8,
                                                tag="padq",
                                                name=f"pq_{g}_{hp}_{p}")
                        nc.vector.tensor_add(padq[p][:, i, :],
                                             pjt[:, 0, :], pjt[:, 1, :])
                        if i == 1:
                            pend_sums.append(("dr", 0, padq[p], 0))
                        pend_pv.append((2 * p, pjt, i, (0, 0)))
                        fill()
                # diagonal pairs: j = 4g+2dp, 4g+2dp+1
                for dp in range(2):
                    for i, h in enumerate(heads):
                        j0 = 4 * g + 2 * dp
                        c00, c01 = 256 * dp, 256 * dp + 128
                        st2 = pst2.tile([128, 2, CHUNK], f32, tag="pst2",
                                        name=f"std_{g}_{hp}_{dp}_{i}")
                        nc.tensor.matmul(st2[:, 0, c00:],
                                         kT[:, j0 * 128:(j0 + 1) * 128],
                                         qT[:, h, q0 + c00:q0 + CHUNK],
                                         start=True, stop=True,
                                         skip_group_check=True)
                        nc.tensor.matmul(st2[:, 1, c01:],
                                         kT[:, (j0 + 1) * 128:(j0 + 2) * 128],
                                         qT[:, h, q0 + c01:q0 + CHUNK],
                                         start=True, stop=True,
                                         skip_group_check=True)
                        drain()
                        pjt = pjp.tile([128, 2, CHUNK], bf16, tag="pj",
                                       name=f"pjd_{g}_{hp}_{dp}_{i}")
                        nc.scalar.activation(out=pjt[:, 0, c00:],
                                             in_=st2[:, 0, c00:],
                                             func=exp_, bias=ebias[:],
                                             scale=1.0)
                        nc.scalar.activation(out=pjt[:, 1, c01:],
                                             in_=st2[:, 1, c01:],
                                             func=exp_, bias=ebias[:],
                                             scale=1.0)
                        nc.vector.memset(pjt[:, 1, c00:c01], 0.0)
                        for jj, cc in ((0, c00), (1, c01)):
                            nc.gpsimd.affine_select(
                                out=pjt[:, jj, cc:cc + 128],
                                in_=pjt[:, jj, cc:cc + 128],
                                pattern=[[1, 128]],
                                compare_op=mybir.AluOpType.is_ge,
                                fill=0.0,
                                base=0,
                                channel_multiplier=-1)
                        pdd = paddp.tile([128, CHUNK], bf16, tag="padd",
                                         name=f"pd_{g}_{hp}_{dp}_{i}")
                        nc.vector.tensor_add(pdd[:, c00:], pjt[:, 0, c00:],
                                             pjt[:, 1, c00:])
                        pend_sums.append(("last" if dp == 1 else "bf",
                                          i, pdd, c00))
                        pend_pv.append((j0, pjt, i, (c00, c01)))
                        fill()
                drain(0, 0)
                # finalize: r64 = 4/denominator; y8/dy8 split for oproj
                for i, h in enumerate(heads):
                    r64 = atmp.tile([64, CHUNK], f32, tag="r64",
                                    name=f"r_{g}_{hp}_{i}")
                    nc.vector.reciprocal(r64[:], sms[i])
                    t = ttp.tile([128, CHUNK], f32, tag="t",
                                 name=f"t_{g}_{hp}_{i}")
                    for half in range(2):
                        p0 = 64 * half
                        tsl = t[p0:p0 + 64, :]
                        nc.vector.tensor_mul(tsl, ys[i][p0:p0 + 64, :],
                                             r64[:])
                        y8sl = y8t[p0:p0 + 64, h, :]
                        if g == 3:
                            # tail: DVE is idle and faster than the Q7 chain
                            nc.vector.tensor_copy(out=y8sl, in_=tsl)
                            nc.vector.tensor_sub(dy8t[p0:p0 + 64, h, :],
                                                 tsl, y8sl)
                        else:
                            nc.gpsimd.tensor_copy(out=y8sl, in_=tsl)
                            nc.gpsimd.tensor_sub(dy8t[p0:p0 + 64, h, :],
                                                 tsl, y8sl)
                    fill()
                while fi[0] < len(fillers):
                    fillers[fi[0]]()
                    fi[0] += 1

            def oproj_units(g, alt=False, split_store=False):
                """16 oproj unit closures + final store; 3-term fp8
                DoubleRow over head pairs. alt: alternate pox/py pools
                (tail group, attention pools free)."""
                obuf = obp.tile([128, 4, D], bf16, tag="obuf",
                                name=f"ob_{g}")
                y8t, dy8t = y8s[g]
                units = []

                def unit(tb, oc, n):
                    def go():
                        pool = py if (alt and n % 2 == 1) else pox
                        po = pool.tile([128, CHUNK], f32,
                                       tag="py" if (alt and n % 2 == 1)
                                       else "pox",
                                       name=f"op_{g}_{tb}_{oc}")
                        first, last = True, None
                        mms = []
                        for hp in range(2):
                            h0 = 2 * hp
                            for term in range(3):
                                ysl = dy8t if term == 1 else y8t
                                w_i = 1 if term == 2 else 0
                                mms.append((
                                    ysl[:, h0:h0 + 2,
                                        tb * 128:(tb + 1) * 128],
                                    ow_s[:, w_i, h0:h0 + 2,
                                         oc * CHUNK:(oc + 1) * CHUNK]))
                        for mi, (lhsT, rhs) in enumerate(mms):
                            nc.tensor.matmul(po[:], lhsT, rhs,
                                             start=(mi == 0),
                                             stop=(mi == len(mms) - 1),
                                             perf_mode=DR,
                                             skip_group_check=True)
                        dst = obuf[:, tb, oc * CHUNK:(oc + 1) * CHUNK]
                        if n % 3 != 2:
                            nc.scalar.activation(out=dst, in_=po[:],
                                                 func=copy_,
                                                 scale=1.0 / 128.0)
                        else:
                            nc.vector.tensor_scalar_mul(dst, po[:],
                                                        1.0 / 128.0)
                        if split_store and oc == 3:
                            nc.sync.dma_start(outd[g, :, tb, :],
                                              obuf[:, tb, :])
                    return go

                n = 0
                for tb in range(4):
                    for oc in range(4):
                        units.append(unit(tb, oc, n))
                        n += 1
                if not split_store:
                    def store():
                        nc.sync.dma_start(outd[g], obuf[:])
                    units.append(store)
                return units

            # ---- schedule ----
            xt0 = emit_xdmas(0, first=True)
            xt1 = proj_chunk(0, xt0, nxt=True)
            xt2 = proj_chunk(1, xt1, nxt=True)
            xt3 = proj_chunk(2, xt2, nxt=True)
            attn_pass(0, 0)
            proj_chunk(3, xt3)
            attn_pass(0, 1)
            attn_pass(1, 0, fillers=oproj_units(0))
            attn_pass(1, 1)
            attn_pass(2, 0, fillers=oproj_units(1))
            attn_pass(2, 1)
            attn_pass(3, 0, fillers=oproj_units(2))
            attn_pass(3, 1)
            for u in oproj_units(3, alt=True, split_store=True):
                u()

    nc.compile()
    return nc


_CACHED = {}
LAST_EXEC_NS = None


def _run(nc, in_maps, **kwargs):
    from concourse.bass_utils import run_bass_kernel_spmd
    return run_bass_kernel_spmd(nc, in_maps, core_ids=list(range(NCORES)),
                                **kwargs)


def _make_in_maps(x, qw, kw, vw, ow, qg):
    import ml_dtypes
    bf = ml_dtypes.bfloat16
    f8 = ml_dtypes.float8_e4m3

    def split8(a, scale):
        a32 = np.asarray(a, np.float32) * np.float32(scale)
        a8 = a32.astype(f8)
        da = (a32 - a8.astype(np.float32)).astype(f8)
        return np.stack([a8, da])  # [2, ...]

    cosT, sinT = _rope_tables()
    cossin = np.concatenate([cosT, sinT], axis=0).astype(bf)   # [128, T]
    sincos = np.concatenate([sinT, cosT], axis=0).astype(bf)   # [128, T]

    def xlayout(xb):
        s = split8(xb.reshape(T, D).T, 1.0)        # [2, D, T]
        return np.ascontiguousarray(
            s.reshape(2, 4, 4, 128, T).transpose(1, 0, 2, 3, 4))

    xcats = [xlayout(x[b]) for b in range(B)]

    in_maps = []
    for c in range(NCORES):
        b, kvg = c // 4, c % 4
        h0 = HPC * kvg
        def pmajor(a, m):
            # [2, D_or_512, m] -> [128, 2, ko, m] partition-major
            return np.ascontiguousarray(
                a.reshape(2, -1, 128, m).transpose(2, 0, 1, 3))

        qwcat = pmajor(split8(qw[h0 * HD:(h0 + HPC) * HD, :].T, WSCALE), 512)
        kwcat = pmajor(split8(kw[kvg * HD:(kvg + 1) * HD, :].T, WSCALE), HD)
        vwcat = pmajor(split8(vw[kvg * HD:(kvg + 1) * HD, :].T, WSCALE), HD)
        owcat = pmajor(split8(ow[:, h0 * HD:(h0 + HPC) * HD].T, WSCALE), D)
        # s_i folds qg gain and 1/sqrt(HD); 32x psum scale cancels in the
        # ratio, only the EPS bias needs the 32^2 factor.
        s = np.array([qg[h0] / np.sqrt(HD), qg[h0 + 1] / np.sqrt(HD),
                      qg[h0 + 2] / np.sqrt(HD), qg[h0 + 3] / np.sqrt(HD),
                      1.0], np.float32)
        normo = np.broadcast_to(
            (1.0 / (HD * s * s))[None, :, None], (128, 5, 128)
        ).astype(np.float32).copy()
        normb = np.broadcast_to(
            (EPS * WSCALE * WSCALE / (s * s))[None, :],
            (128, 5)).astype(np.float32).copy()
        in_maps.append({
            "xcat": xcats[b],
            "qwcat": np.ascontiguousarray(qwcat),
            "kwcat": np.ascontiguousarray(kwcat),
            "vwcat": np.ascontiguousarray(vwcat),
            "owcat": np.ascontiguousarray(owcat),
            "csd": cossin, "csd2": sincos,
            "normod": normo, "normbd": normb,
        })
    return in_maps


def kernel(x, qw, kw, vw, ow, qg):
    global LAST_EXEC_NS
    x = np.ascontiguousarray(x, dtype=np.float32)
    qw = np.asarray(qw, dtype=np.float32)
    kw = np.asarray(kw, dtype=np.float32)
    vw = np.asarray(vw, dtype=np.float32)
    ow = np.asarray(ow, dtype=np.float32)
    qg = np.asarray(qg, dtype=np.float32)

    if "nc" not in _CACHED:
        _CACHED["nc"] = _build_program()
    nc = _CACHED["nc"]

    in_maps = _make_in_maps(x, qw, kw, vw, ow, qg)
    res = _run(nc, in_maps)
    LAST_EXEC_NS = res.exec_time_ns
    out = np.empty((B, T, D), np.float32)
    for b in range(B):
        acc = np.zeros((4, 128, 4, D), np.float32)
        for kvg in range(4):
            acc += res.results[4 * b + kvg]["o"].astype(np.float32)
        # [g, p, tb, d] -> rows g*512 + tb*128 + p
        out[b] = acc.transpose(0, 2, 1, 3).reshape(T, D)
    return np.ascontiguousarray(out)
